# revision 3
# baseline (speedup 1.0000x reference)
"""ConstituentAttention Trainium2 kernel.

Math (derived from the reference):
  - score is masked to the super/sub-diagonal only, so the row softmax is a
    2-element softmax: a_u[i] = sigmoid((s_u[i]-s_l[i])/E), a_l = 1-a_u,
    where s_u[i] = q_i.k_{i+1}, s_l[i] = q_i.k_{i-1}.
  - neighbor_attn = prior + (1-prior)*g where g == sqrt(1e-9) =: C0 everywhere
    except g[i,i+1] = g[i+1,i] = sqrt(a_u[i]*a_l[i+1] + 1e-9) =: g_u[i].
  - log-space prefix products collapse to c_attn[i,j] = exp(-|U[j]-U[i]|) for
    i != j, where U = exclusive prefix sum of u_i = log(na[i,i+1] + 1e-9);
    diagonal of c_attn = na[i,i].

Sharding: data-parallel over batch, one batch element per NeuronCore (B=8).

Layout notes: i = r*128 + p (partition p fast, block r = 0..7 slow), so the
per-index arrays live as [128, 8] SBUF tiles.  s_u/s_l are extracted from
[1, S]-ish linear staging rows with THREE free-dim offsets (i-1, i, i+1)
stacked as [128, 24] tiles, which turns every partition-shift the algorithm
needs into a free-dim offset.
"""

import numpy as np

import concourse.bass as bass
import concourse.tile as tile
from concourse import mybir
from concourse.bass_utils import run_bass_kernel_spmd

S, B, E, P = 1024, 8, 512, 64
P2 = 2 * P
NB = S // 128
C0 = float(np.sqrt(1e-9))
NEG = -1e30
F32 = mybir.dt.float32
F32R = mybir.dt.float32r
AF = mybir.ActivationFunctionType
ALU = mybir.AluOpType

_CACHE = {}


def _ap(handle_or_ap, offset, dims):
    a0 = handle_or_ap[:] if not isinstance(handle_or_ap, bass.AP) else handle_or_ap
    return bass.AP(tensor=a0.tensor, offset=offset, ap=[list(d) for d in dims])


def _r(ap):
    return ap.bitcast(F32R)


def _split_multi_waits(nc):
    """This toolchain's walrus accepts at most ONE embedded on_wait per
    instruction; hoist extras into standalone EventSemaphore waits just
    before the instruction on the same engine."""
    n = 0
    for bb in nc.main_func.blocks:
        new = []
        for ins in bb.instructions:
            si = ins.sync_info
            if si is not None and si.on_wait and len(si.on_wait) > 1:
                for w in si.on_wait[:-1]:
                    n += 1
                    wi = mybir.InstEventSemaphore(
                        name=f"I-waitsplit-{n}",
                        opcode="EventSemaphore",
                        engine=ins.engine,
                        sync_info=mybir.SyncInfo(on_wait=[w], on_update=[]),
                    )
                    try:
                        nc.register_instruction(wi)
                    except Exception:
                        pass
                    new.append(wi)
                si.on_wait = si.on_wait[-1:]
            new.append(ins)
        try:
            bb.instructions[:] = new
        except TypeError:
            bb.instructions = new
    return n


def build_nc():
    nc = bass.Bass()

    xT = nc.dram_tensor("xT", [E, S], F32, kind="ExternalInput")
    wT = nc.dram_tensor("wT", [E, P2], F32, kind="ExternalInput")
    bvec = nc.dram_tensor("bvec", [P2, 1], F32, kind="ExternalInput")
    prior = nc.dram_tensor("prior", [S, S], F32, kind="ExternalInput")
    na_out = nc.dram_tensor("na_out", [S, S], F32, kind="ExternalOutput")
    c_out = nc.dram_tensor("c_out", [S, S], F32, kind="ExternalOutput")

    # Window masks are [128,130]: for row-block r the band lives in absolute
    # cols [r*128-1, r*128+129); with window origin w0 = r*128-1 the super-diag
    # sits at rel col p+2, diag at p+1, sub-diag at p, independent of r.
    p_i = np.arange(128)[:, None]
    c_i = np.arange(130)[None, :]
    mu_h = nc.inline_tensor((c_i == p_i + 2).astype(np.float32), "mask_u")
    ml_h = nc.inline_tensor((c_i == p_i).astype(np.float32), "mask_l")
    md_h = nc.inline_tensor((c_i == p_i + 1).astype(np.float32), "mask_d")
    m1d_h = nc.inline_tensor((c_i != p_i + 1).astype(np.float32), "mask_1md")
    # lhsT for within-block inclusive cumsum over partitions: out = triu.T @ u
    triu_h = nc.inline_tensor(
        np.triu(np.ones((128, 128), np.float32)), "triu_ones"
    )
    ones_col_h = nc.inline_tensor(np.ones((128, 1), np.float32), "ones_col")
    ones_row_h = nc.inline_tensor(np.ones((1, 128), np.float32), "ones_row")

    with tile.TileContext(nc) as tc:
        with (
            tc.tile_pool(name="setup", bufs=1) as setup,
            tc.tile_pool(name="blk", bufs=3) as blk,
            tc.tile_pool(name="prp", bufs=8) as prp,
            tc.tile_pool(name="mm", bufs=2, space="PSUM") as mm,
            tc.tile_pool(name="mm1", bufs=2, space="PSUM") as mm1,
            tc.tile_pool(name="ps_small", bufs=2, space="PSUM") as ps_small,
            tc.tile_pool(name="psrep", bufs=1, space="PSUM") as psrep,
            tc.tile_pool(name="dram", bufs=1, space="DRAM") as dram,
        ):
            # -------- critical-path loads first on SP: xT chunks + wT -------
            xT_t = setup.tile([128, 4, S], F32)
            wT_t = setup.tile([128, 4, P2], F32)
            bias_t = setup.tile([128, 1], F32)
            nc.sync.dma_start(
                out=xT_t[:, 0, :], in_=_ap(xT, 0, [[S, 128], [1, S]]))
            nc.sync.dma_start(
                out=wT_t,
                in_=_ap(wT, 0, [[P2, 128], [128 * P2, 4], [1, P2]]))
            nc.sync.dma_start(out=bias_t, in_=bvec[:])
            for c in range(1, 4):
                nc.sync.dma_start(
                    out=xT_t[:, c, :],
                    in_=_ap(xT, c * 128 * S, [[S, 128], [1, S]]))

            # ------- prior band gathers (early; feed U chain + c diag) ------
            pr_u = setup.tile([128, NB], F32)            # prior[i, i+1]
            nc.vector.memset(pr_u[:, 7:8], 0.0)  # row 127 stays 0: no (1023,1024)
            nc.gpsimd.dma_start(
                out=pr_u[:, 0:7],
                in_=_ap(prior, 1, [[S + 1, 128], [128 * (S + 1), 7]]))
            nc.gpsimd.dma_start(
                out=pr_u[0:127, 7:8],
                in_=_ap(prior, 896 * (S + 1) + 1, [[S + 1, 127], [1, 1]]))
            pr_d = setup.tile([128, NB], F32)            # prior[i, i]
            nc.gpsimd.dma_start(
                out=pr_d, in_=_ap(prior, 0, [[S + 1, 128], [128 * (S + 1), 8]]))

            # constants via the Act queue (fast HWDGE; Act idle this early)
            mu_t = setup.tile([128, 130], F32)
            nc.scalar.dma_start(out=mu_t, in_=mu_h[:])
            ml_t = setup.tile([128, 130], F32)
            nc.scalar.dma_start(out=ml_t, in_=ml_h[:])
            md_t = setup.tile([128, 130], F32)
            nc.scalar.dma_start(out=md_t, in_=md_h[:])
            m1d_t = setup.tile([128, 130], F32)
            nc.scalar.dma_start(out=m1d_t, in_=m1d_h[:])
            triu_t = setup.tile([128, 128], F32)
            nc.scalar.dma_start(out=triu_t, in_=triu_h[:])
            ones_col = setup.tile([128, 1], F32)
            nc.scalar.dma_start(out=ones_col, in_=ones_col_h[:])
            ones_row = setup.tile([1, 128], F32)
            nc.scalar.dma_start(out=ones_row, in_=ones_row_h[:])

            # preload the Sigmoid activation table during the idle head
            eps_t = setup.tile([128, 1], F32)
            nc.vector.memset(eps_t, 1e-9)
            warm_t = setup.tile([1, 1], F32)
            nc.scalar.activation(warm_t, eps_t[0:1, 0:1], AF.Sigmoid)

            # PE clock warmup: ~3us of continuous dummy matmuls (results
            # are garbage; the Ur broadcast later overwrites this PSUM bank).
            warm_ps = psrep.tile([128, S], F32, tag="urep")
            for d in range(8):
                nc.tensor.matmul(warm_ps[0:64, 0:128],
                                 lhsT=wT_t[:, 0, 0:64],
                                 rhs=wT_t[:, 0, :],
                                 start=True, stop=True)

            # ---------------- qT/kT = (x @ W.T).T halves  [64, S] ----------
            # fp32r matmuls: out free 512 >= 256 -> 1 cycle/row.  The product
            # chain is split at col 511 and interleaved with the j-halves so
            # band extraction for cols [0,511) overlaps the j=1 matmuls.
            qT_t = setup.tile([64, S], F32)
            kT_t = setup.tile([64, S], F32)
            su_st = setup.tile([1, 1026], F32)
            sl_st = setup.tile([1, 1026], F32)
            tu_t = setup.tile([64, S - 1], F32)
            tl_t = setup.tile([64, S - 1], F32)
            with tc.high_priority():
                nc.vector.memset(su_st[:, 0:1], NEG)
                nc.vector.memset(su_st[:, 1024:1026], NEG)
                nc.vector.memset(sl_st[:, 0:2], NEG)
                nc.vector.memset(sl_st[:, 1025:1026], NEG)
                for j in range(2):
                    for half, dest_t in enumerate((qT_t, kT_t)):
                        ps = mm.tile([64, 512], F32, tag="mmbig")
                        for c in range(4):
                            nc.tensor.matmul(
                                ps[:],
                                lhsT=wT_t[:, c, half * 64:(half + 1) * 64],
                                rhs=xT_t[:, c, j * 512:(j + 1) * 512],
                                start=(c == 0),
                                stop=(c == 3),
                            )
                        if half == 0:
                            nc.vector.tensor_scalar_add(
                                dest_t[:, j * 512:(j + 1) * 512], ps,
                                bias_t[0:64, 0:1])
                        else:
                            nc.scalar.activation(
                                dest_t[:, j * 512:(j + 1) * 512], ps,
                                AF.Identity, bias=bias_t[64:128, 0:1])
                    # band products for the cols this j-half completes:
                    #   su_stage[k] = s_u[k-1] = tu[k-1]  (s_u[1023] = -inf)
                    #   sl_stage[k] = s_l[k-1] = tl[k-2]  (s_l[0] = -inf)
                    lo, hi = (0, 511) if j == 0 else (511, 1023)
                    w = hi - lo
                    nc.vector.tensor_mul(tu_t[:, lo:hi], qT_t[:, lo:hi],
                                         kT_t[:, lo + 1:hi + 1])
                    nc.vector.tensor_mul(tl_t[:, lo:hi], qT_t[:, lo + 1:hi + 1],
                                         kT_t[:, lo:hi])
                    for src_t, st_t, off, on_dve in ((tu_t, su_st, 1, True),
                                                     (tl_t, sl_st, 2, False)):
                        ps1 = mm1.tile([1, 512], F32, tag="ones")
                        nc.tensor.matmul(ps1[0:1, 0:w],
                                         lhsT=ones_col[0:64, :],
                                         rhs=src_t[:, lo:hi],
                                         start=True, stop=True)
                        if on_dve:
                            nc.vector.tensor_copy(st_t[:, off + lo:off + hi],
                                                  ps1[0:1, 0:w])
                        else:
                            nc.scalar.activation(st_t[:, off + lo:off + hi],
                                                 ps1[0:1, 0:w], AF.Copy)

            # [128, 24] stacks: col groups g=0,1,2 hold offsets i-1,i,i+1
            with tc.high_priority():
                su_d = dram.tile([1026], F32)
                nc.scalar.dma_start(out=su_d[:], in_=su_st)
                sl_d = dram.tile([1026], F32)
                nc.scalar.dma_start(out=sl_d[:], in_=sl_st)
                s_uu = setup.tile([128, 3, NB], F32)
                s_ll = setup.tile([128, 3, NB], F32)
                for g in range(3):
                    nc.scalar.dma_start(
                        out=s_uu[:, g, :],
                        in_=_ap(su_d[:], g, [[1, 128], [128, NB]]))
                    nc.scalar.dma_start(
                        out=s_ll[:, g, :],
                        in_=_ap(sl_d[:], g, [[1, 128], [128, NB]]))

                # 2-element softmax via sigmoid on all 3 offset groups at once
                diff_t = setup.tile([128, 3, NB], F32)
                nc.vector.tensor_sub(diff_t, s_uu, s_ll)
                a_u = setup.tile([128, 3, NB], F32)
                nc.scalar.activation(a_u, diff_t, AF.Sigmoid, scale=1.0 / E)
                a_l = setup.tile([128, 3, NB], F32)
                nc.scalar.activation(a_l, diff_t, AF.Sigmoid, scale=-1.0 / E)

                # g_l[i] = g_u[i-1] = sqrt(a_u[i-1]*a_l[i] + eps)  (cols 0:8)
                # g_u[i]            = sqrt(a_u[i]*a_l[i+1] + eps)  (cols 8:16)
                gq_t = setup.tile([128, 2, NB], F32)
                nc.vector.tensor_mul(gq_t, _ap(a_u[:], 0, [[24, 128], [8, 2], [1, NB]]),
                                     _ap(a_l[:], 8, [[24, 128], [8, 2], [1, NB]]))
                g_t = setup.tile([128, 2, NB], F32)
                nc.scalar.activation(g_t, gq_t, AF.Sqrt, bias=eps_t[:, 0:1])
                g_l = g_t[:, 0, :]
                g_u = g_t[:, 1, :]

                # gu_c/gl_c = g - C0 for the banded block-loop update
                gc_t = setup.tile([128, 2, NB], F32)
                nc.vector.tensor_scalar_sub(gc_t, g_t, C0)
                gl_c = gc_t[:, 0, :]
                gu_c = gc_t[:, 1, :]

                # na[i,i+1] = g_u + pr_u*(1-g_u);  u = ln(na + eps)
                omg_t = setup.tile([128, NB], F32)
                nc.vector.tensor_scalar(omg_t, g_u, -1.0, 1.0, op0=ALU.mult,
                                        op1=ALU.add)
                t_tmp = setup.tile([128, NB], F32)
                nc.vector.tensor_mul(t_tmp, pr_u, omg_t)
                na_bu = setup.tile([128, NB], F32)
                nc.vector.tensor_add(na_bu, t_tmp, g_u)
                u_t = setup.tile([128, NB], F32)
                nc.scalar.activation(u_t, na_bu, AF.Ln, bias=eps_t[:, 0:1])
                nd_t = setup.tile([128, NB], F32)            # na[i, i]
                nc.scalar.activation(nd_t, pr_d, AF.Copy, bias=C0, scale=1.0 - C0)

                # ---- U = exclusive prefix sum of u (no DRAM round trips) ----
                inc_ps = ps_small.tile([128, NB], F32, tag="tiny")
                nc.tensor.matmul(inc_ps, lhsT=triu_t, rhs=u_t, start=True, stop=True)
                exc_t = setup.tile([128, NB], F32)
                nc.vector.tensor_sub(exc_t, inc_ps, u_t)

                cs_ps = ps_small.tile([1, NB], F32, tag="tiny")   # per-block sums
                nc.tensor.matmul(cs_ps, lhsT=ones_col, rhs=u_t, start=True, stop=True)
                bp_t = setup.tile([1, NB], F32)
                nc.vector.memset(bp_t[:, 0:1], 0.0)
                nc.vector.tensor_copy(bp_t[:, 1:8], cs_ps[0:1, 0:7])
                zer_t = setup.tile([1, NB], F32)
                nc.vector.memset(zer_t, 0.0)
                bpx_t = setup.tile([1, NB], F32)             # exclusive block prefix
                nc.vector.tensor_tensor_scan(bpx_t, bp_t, zer_t, 0.0,
                                             op0=ALU.add, op1=ALU.add)
                bpr_ps = ps_small.tile([128, NB], F32, tag="tiny")
                nc.tensor.matmul(bpr_ps, lhsT=ones_row, rhs=bpx_t, start=True,
                                 stop=True)
                U_t = setup.tile([128, NB], F32)
                nc.vector.tensor_add(U_t, exc_t, bpr_ps)

                # U_rep[p, j] = U[j] via SBUF reshape DMA + ones broadcast matmul
                U_d = dram.tile([S], F32)
                nc.scalar.dma_start(out=_ap(U_d[:], 0, [[1, 128], [128, NB]]),
                                    in_=U_t)
                U_lin = setup.tile([1, S], F32)
                nc.scalar.dma_start(out=U_lin, in_=U_d[:])
                Ur_ps = psrep.tile([128, S], F32, tag="urep")
                for lo in (0, 512):
                    nc.tensor.matmul(Ur_ps[:, lo:lo + 512], lhsT=ones_row,
                                     rhs=U_lin[0:1, lo:lo + 512], start=True,
                                     stop=True)
                Ur_sb = setup.tile([128, S], F32)
                nc.scalar.activation(Ur_sb, Ur_ps, AF.Copy)

            # ---------------- pass 1: prior loads + na full rows ----------
            pr_ts, na_ts = [], []
            for r in range(NB):
                pr_t = prp.tile([128, S], F32, tag="pr")
                nc.sync.dma_start(out=pr_t, in_=prior[r * 128:(r + 1) * 128, :])
                pr_ts.append(pr_t)
            for r in range(NB):
                na_t = blk.tile([128, S], F32, tag="na")
                nc.gpsimd.tensor_scalar(na_t, pr_ts[r], 1.0 - C0, C0,
                                        op0=ALU.mult, op1=ALU.add)
                nc.sync.dma_start(out=na_out[r * 128:(r + 1) * 128, :],
                                  in_=na_t)
                na_ts.append(na_t)

            # ---------------- pass 1b: na band windows ----------------
            for r in range(NB):
                w0 = r * 128 - 1
                wlo = max(w0, 0)
                whi = min(w0 + 130, S)
                wd = whi - wlo
                mo = wlo - w0
                pr_t = pr_ts[r]

                # band window: g = C0 + M_u*(g_u-C0) + M_l*(g_l-C0)
                gwin = blk.tile([128, 130], F32, tag="gwin")
                nc.vector.tensor_scalar(gwin[:, :wd], mu_t[:, mo:mo + wd],
                                        gu_c[:, r:r + 1], C0,
                                        op0=ALU.mult, op1=ALU.add)
                t2w = blk.tile([128, 130], F32, tag="t2w")
                nc.vector.tensor_scalar(t2w[:, :wd], ml_t[:, mo:mo + wd],
                                        gl_c[:, r:r + 1], None, op0=ALU.mult)
                gw2 = blk.tile([128, 130], F32, tag="gw2")
                nc.vector.tensor_add(gw2[:, :wd], gwin[:, :wd], t2w[:, :wd])
                # na_win = g + prior*(1-g) = g + prior - prior*g
                t3w = blk.tile([128, 130], F32, tag="t3w")
                nc.vector.tensor_mul(t3w[:, :wd], pr_t[:, wlo:whi],
                                     gw2[:, :wd])
                t4w = blk.tile([128, 130], F32, tag="t4w")
                nc.vector.tensor_sub(t4w[:, :wd], pr_t[:, wlo:whi],
                                     t3w[:, :wd])
                naw = blk.tile([128, 130], F32, tag="naw")
                nc.vector.tensor_add(naw[:, :wd], t4w[:, :wd], gw2[:, :wd])
                nc.sync.dma_start(
                    out=_ap(na_out, r * 128 * S + wlo, [[S, 128], [1, wd]]),
                    in_=naw[:, :wd])

            # ---------------- pass 2: c_attn rows ----------------
            for r in range(NB):
                w0 = r * 128 - 1
                wlo = max(w0, 0)
                whi = min(w0 + 130, S)
                wd = whi - wlo
                mo = wlo - w0
                veng = nc.vector if r % 2 == 0 else nc.gpsimd

                # c block: exp(-|U[j] - U[i]|), diag <- na[i,i]
                cs_t = blk.tile([128, S], F32, tag="cs")
                veng.tensor_scalar(cs_t, Ur_sb, U_t[:, r:r + 1], None,
                                   op0=ALU.subtract)
                cn_t = blk.tile([128, S], F32, tag="cn")
                veng.tensor_scalar(cn_t, cs_t, -1.0, None, op0=ALU.mult)
                cd_t = blk.tile([128, S], F32, tag="cd")
                nc.vector.tensor_max(cd_t, cs_t, cn_t)
                c2_t = blk.tile([128, S], F32, tag="c2")
                nc.scalar.activation(c2_t, cd_t, AF.Exp, scale=-1.0)
                t5w = blk.tile([128, 130], F32, tag="t5w")
                nc.vector.tensor_scalar(t5w[:, :wd], md_t[:, mo:mo + wd],
                                        nd_t[:, r:r + 1], None, op0=ALU.mult)
                t6w = blk.tile([128, 130], F32, tag="t6w")
                nc.vector.tensor_mul(t6w[:, :wd], c2_t[:, wlo:whi],
                                     m1d_t[:, mo:mo + wd])
                nc.vector.tensor_add(c2_t[:, wlo:whi], t5w[:, :wd],
                                     t6w[:, :wd])

                nc.sync.dma_start(out=c_out[r * 128:(r + 1) * 128, :],
                                  in_=c2_t)

    _split_multi_waits(nc)
    return nc


def _get_nc():
    if "nc" not in _CACHE:
        _CACHE["nc"] = build_nc()
    return _CACHE["nc"]


def run(inputs, trace=False, tmpdir=None):
    nc = _get_nc()
    context = np.asarray(inputs["context"], np.float32)
    prior = np.asarray(inputs["prior"], np.float32)
    w = np.asarray(inputs["proj_weight"], np.float32)
    bias = np.asarray(inputs["proj_bias"], np.float32)

    wT = np.ascontiguousarray(w.T)                     # [E, 2P]
    bcol = np.ascontiguousarray(bias.reshape(P2, 1))
    in_maps = []
    for b in range(B):
        in_maps.append({
            "xT": np.ascontiguousarray(context[:, b, :].T),   # [E, S]
            "wT": wT,
            "bvec": bcol,
            "prior": np.ascontiguousarray(prior[b]),
        })
    try:
        res = run_bass_kernel_spmd(nc, in_maps, list(range(B)), trace=trace,
                                   tmpdir=tmpdir)
    except ModuleNotFoundError:
        res = run_bass_kernel_spmd(nc, in_maps, list(range(B)), trace=False)
    c = np.stack([res.results[i]["c_out"] for i in range(B)])
    na = np.stack([res.results[i]["na_out"] for i in range(B)])
    return (c, na), res


def kernel(**inputs):
    (c, na), _ = run(inputs)
    return (c, na)



# revision 5
# speedup vs baseline: 1.8678x; 1.8678x over previous
"""ConstituentAttention Trainium2 kernel.

Math (derived from the reference):
  - score is masked to the super/sub-diagonal only, so the row softmax is a
    2-element softmax: a_u[i] = sigmoid((s_u[i]-s_l[i])/E), a_l = 1-a_u,
    where s_u[i] = q_i.k_{i+1}, s_l[i] = q_i.k_{i-1}.
  - neighbor_attn = prior + (1-prior)*g where g == sqrt(1e-9) =: C0 everywhere
    except g[i,i+1] = g[i+1,i] = sqrt(a_u[i]*a_l[i+1] + 1e-9) =: g_u[i].
  - log-space prefix products collapse to c_attn[i,j] = exp(-|U[j]-U[i]|) for
    i != j, where U = exclusive prefix sum of u_i = log(na[i,i+1] + 1e-9);
    diagonal of c_attn = na[i,i].

Sharding: data-parallel over batch, one batch element per NeuronCore (B=8).

Engine discipline (the v1 lesson): DVE 2-port ops and GpSimd take an
exclusive lock on the shared SBUF port pair - concurrent DVE+GpSimd work
stretches BOTH by ~15x.  So: all elementwise compute lives on Vector, all
activations on Scalar, GpSimd only issues the early scatter-gather DMAs.

Band handling: the tri-diagonal na values are computed as tiny [128, NB]
vectors and written with narrow 3-elements-per-row strided stores AFTER the
bulk rows (same HWDGE ring -> FIFO gives WAW order).  The c diagonal is
patched pre-exp in SBUF: cd[i,i] -= ln(na[i,i]) so exp(-cd) lands na[i,i].
"""

import numpy as np

import concourse.bass as bass
import concourse.tile as tile
from concourse import mybir
from concourse.bass_utils import run_bass_kernel_spmd

S, B, E, P = 1024, 8, 512, 64
P2 = 2 * P
NB = S // 128
C0 = float(np.sqrt(1e-9))
NEG = -1e30
F32 = mybir.dt.float32
F32R = mybir.dt.float32r
AF = mybir.ActivationFunctionType
ALU = mybir.AluOpType

_CACHE = {}


def _ap(handle_or_ap, offset, dims):
    a0 = handle_or_ap[:] if not isinstance(handle_or_ap, bass.AP) else handle_or_ap
    return bass.AP(tensor=a0.tensor, offset=offset, ap=[list(d) for d in dims])


def _split_multi_waits(nc):
    """This toolchain's walrus accepts at most ONE embedded on_wait per
    instruction; hoist extras into standalone EventSemaphore waits just
    before the instruction on the same engine."""
    n = 0
    for bb in nc.main_func.blocks:
        new = []
        for ins in bb.instructions:
            si = ins.sync_info
            if si is not None and si.on_wait and len(si.on_wait) > 1:
                for w in si.on_wait[:-1]:
                    n += 1
                    wi = mybir.InstEventSemaphore(
                        name=f"I-waitsplit-{n}",
                        opcode="EventSemaphore",
                        engine=ins.engine,
                        sync_info=mybir.SyncInfo(on_wait=[w], on_update=[]),
                    )
                    try:
                        nc.register_instruction(wi)
                    except Exception:
                        pass
                    new.append(wi)
                si.on_wait = si.on_wait[-1:]
            new.append(ins)
        try:
            bb.instructions[:] = new
        except TypeError:
            bb.instructions = new
    return n


def build_nc():
    nc = bass.Bass()

    xT = nc.dram_tensor("xT", [E, S], F32, kind="ExternalInput")
    wT = nc.dram_tensor("wT", [E, P2], F32, kind="ExternalInput")
    bvec = nc.dram_tensor("bvec", [P2, 1], F32, kind="ExternalInput")
    prior = nc.dram_tensor("prior", [S, S], F32, kind="ExternalInput")
    na_out = nc.dram_tensor("na_out", [S, S], F32, kind="ExternalOutput")
    c_out = nc.dram_tensor("c_out", [S, S], F32, kind="ExternalOutput")

    # diag mask [128,130]: for row-block r the band lives in absolute cols
    # [r*128-1, r*128+129); with window origin w0 = r*128-1 the diagonal sits
    # at rel col p+1, independent of r.
    p_i = np.arange(128)[:, None]
    c_i = np.arange(130)[None, :]
    md_h = nc.inline_tensor((c_i == p_i + 1).astype(np.float32), "mask_d")
    # lhsT for within-block inclusive cumsum over partitions: out = triu.T @ u
    triu_h = nc.inline_tensor(
        np.triu(np.ones((128, 128), np.float32)), "triu_ones"
    )
    ones_col_h = nc.inline_tensor(np.ones((128, 1), np.float32), "ones_col")
    ones_row_h = nc.inline_tensor(np.ones((1, 128), np.float32), "ones_row")

    with tile.TileContext(nc) as tc:
        with (
            tc.tile_pool(name="setup", bufs=1) as setup,
            tc.tile_pool(name="na", bufs=4) as napool,
            tc.tile_pool(name="cdp", bufs=3) as cdpool,
            tc.tile_pool(name="c2p", bufs=3) as c2pool,
            tc.tile_pool(name="prp", bufs=8) as prp,
            tc.tile_pool(name="mm", bufs=2, space="PSUM") as mm,
            tc.tile_pool(name="mm1", bufs=2, space="PSUM") as mm1,
            tc.tile_pool(name="ps_small", bufs=2, space="PSUM") as ps_small,
            tc.tile_pool(name="psrep", bufs=1, space="PSUM") as psrep,
            tc.tile_pool(name="dram", bufs=1, space="DRAM") as dram,
        ):
            # -------- critical-path loads first on SP: xT chunks + wT -------
            xT_t = setup.tile([128, 4, S], F32)
            wT_t = setup.tile([128, 4, P2], F32)
            bias_t = setup.tile([128, 1], F32)
            nc.sync.dma_start(
                out=xT_t[:, 0, :], in_=_ap(xT, 0, [[S, 128], [1, S]]))
            nc.sync.dma_start(
                out=wT_t,
                in_=_ap(wT, 0, [[P2, 128], [128 * P2, 4], [1, P2]]))
            nc.sync.dma_start(out=bias_t, in_=bvec[:])
            for c in range(1, 4):
                nc.sync.dma_start(
                    out=xT_t[:, c, :],
                    in_=_ap(xT, c * 128 * S, [[S, 128], [1, S]]))

            # ------- prior band gathers (early; feed U chain + band3) ------
            # pr_lu[:, 0, :] = prior[i, i-1] (row 0 unused -> 0)
            # pr_lu[:, 1, :] = prior[i, i+1] (row 1023 unused -> 0)
            pr_lu = setup.tile([128, 2, NB], F32)
            nc.vector.memset(pr_lu[0:1, 0, 0:1], 0.0)
            nc.vector.memset(pr_lu[:, 1, 7:8], 0.0)
            nc.gpsimd.dma_start(
                out=pr_lu[1:128, 0, 0:1],
                in_=_ap(prior, S, [[S + 1, 127], [1, 1]]))
            nc.gpsimd.dma_start(
                out=pr_lu[:, 0, 1:8],
                in_=_ap(prior, 128 * (S + 1) - 1,
                        [[S + 1, 128], [128 * (S + 1), 7]]))
            nc.gpsimd.dma_start(
                out=pr_lu[:, 1, 0:7],
                in_=_ap(prior, 1, [[S + 1, 128], [128 * (S + 1), 7]]))
            nc.gpsimd.dma_start(
                out=pr_lu[0:127, 1, 7:8],
                in_=_ap(prior, 896 * (S + 1) + 1, [[S + 1, 127], [1, 1]]))
            pr_d = setup.tile([128, NB], F32)            # prior[i, i]
            nc.gpsimd.dma_start(
                out=pr_d, in_=_ap(prior, 0, [[S + 1, 128], [128 * (S + 1), 8]]))

            # constants via the Act queue (fast HWDGE; Act idle this early)
            md_t = setup.tile([128, 130], F32)
            nc.scalar.dma_start(out=md_t, in_=md_h[:])
            triu_t = setup.tile([128, 128], F32)
            nc.scalar.dma_start(out=triu_t, in_=triu_h[:])
            ones_col = setup.tile([128, 1], F32)
            nc.scalar.dma_start(out=ones_col, in_=ones_col_h[:])
            ones_row = setup.tile([1, 128], F32)
            nc.scalar.dma_start(out=ones_row, in_=ones_row_h[:])

            # preload the Sigmoid activation table during the idle head
            eps_t = setup.tile([128, 1], F32)
            nc.vector.memset(eps_t, 1e-9)
            warm_t = setup.tile([1, 1], F32)
            nc.scalar.activation(warm_t, eps_t[0:1, 0:1], AF.Sigmoid)

            # PE clock warmup: dummy matmuls (results are garbage; the Ur
            # broadcast later overwrites this PSUM bank).
            warm_ps = psrep.tile([128, S], F32, tag="urep")
            for d in range(8):
                nc.tensor.matmul(warm_ps[0:64, 0:128],
                                 lhsT=wT_t[:, 0, 0:64],
                                 rhs=wT_t[:, 0, :],
                                 start=True, stop=True)

            # ---------------- qT/kT = (x @ W.T).T halves  [64, S] ----------
            # fp32r matmuls: out free 512 >= 256 -> 1 cycle/row.  The product
            # chain is split at col 511 and interleaved with the j-halves so
            # band extraction for cols [0,511) overlaps the j=1 matmuls.
            qT_t = setup.tile([64, S], F32)
            kT_t = setup.tile([64, S], F32)
            su_st = setup.tile([1, 1026], F32)
            sl_st = setup.tile([1, 1026], F32)
            tu_t = setup.tile([64, S - 1], F32)
            tl_t = setup.tile([64, S - 1], F32)
            with tc.high_priority():
                nc.vector.memset(su_st[:, 0:1], NEG)
                nc.vector.memset(su_st[:, 1024:1026], NEG)
                nc.vector.memset(sl_st[:, 0:2], NEG)
                nc.vector.memset(sl_st[:, 1025:1026], NEG)
                for j in range(2):
                    for half, dest_t in enumerate((qT_t, kT_t)):
                        ps = mm.tile([64, 512], F32, tag="mmbig")
                        for c in range(4):
                            nc.tensor.matmul(
                                ps[:],
                                lhsT=wT_t[:, c, half * 64:(half + 1) * 64],
                                rhs=xT_t[:, c, j * 512:(j + 1) * 512],
                                start=(c == 0),
                                stop=(c == 3),
                            )
                        if half == 0:
                            nc.vector.tensor_scalar_add(
                                dest_t[:, j * 512:(j + 1) * 512], ps,
                                bias_t[0:64, 0:1])
                        else:
                            nc.scalar.activation(
                                dest_t[:, j * 512:(j + 1) * 512], ps,
                                AF.Identity, bias=bias_t[64:128, 0:1])
                    # band products for the cols this j-half completes:
                    #   su_stage[k] = s_u[k-1] = tu[k-1]  (s_u[1023] = -inf)
                    #   sl_stage[k] = s_l[k-1] = tl[k-2]  (s_l[0] = -inf)
                    lo, hi = (0, 511) if j == 0 else (511, 1023)
                    w = hi - lo
                    nc.vector.tensor_mul(tu_t[:, lo:hi], qT_t[:, lo:hi],
                                         kT_t[:, lo + 1:hi + 1])
                    nc.vector.tensor_mul(tl_t[:, lo:hi], qT_t[:, lo + 1:hi + 1],
                                         kT_t[:, lo:hi])
                    for src_t, st_t, off, on_dve in ((tu_t, su_st, 1, True),
                                                     (tl_t, sl_st, 2, False)):
                        ps1 = mm1.tile([1, 512], F32, tag="ones")
                        nc.tensor.matmul(ps1[0:1, 0:w],
                                         lhsT=ones_col[0:64, :],
                                         rhs=src_t[:, lo:hi],
                                         start=True, stop=True)
                        if on_dve:
                            nc.vector.tensor_copy(st_t[:, off + lo:off + hi],
                                                  ps1[0:1, 0:w])
                        else:
                            nc.scalar.activation(st_t[:, off + lo:off + hi],
                                                 ps1[0:1, 0:w], AF.Copy)

            # [128, 24] stacks: col groups g=0,1,2 hold offsets i-1,i,i+1
            with tc.high_priority():
                su_d = dram.tile([1026], F32)
                nc.scalar.dma_start(out=su_d[:], in_=su_st)
                sl_d = dram.tile([1026], F32)
                nc.scalar.dma_start(out=sl_d[:], in_=sl_st)
                s_uu = setup.tile([128, 3, NB], F32)
                s_ll = setup.tile([128, 3, NB], F32)
                for g in range(3):
                    nc.scalar.dma_start(
                        out=s_uu[:, g, :],
                        in_=_ap(su_d[:], g, [[1, 128], [128, NB]]))
                    nc.scalar.dma_start(
                        out=s_ll[:, g, :],
                        in_=_ap(sl_d[:], g, [[1, 128], [128, NB]]))

                # 2-element softmax via sigmoid on all 3 offset groups at once
                diff_t = setup.tile([128, 3, NB], F32)
                nc.vector.tensor_sub(diff_t, s_uu, s_ll)
                a_u = setup.tile([128, 3, NB], F32)
                nc.scalar.activation(a_u, diff_t, AF.Sigmoid, scale=1.0 / E)
                a_l = setup.tile([128, 3, NB], F32)
                nc.scalar.activation(a_l, diff_t, AF.Sigmoid, scale=-1.0 / E)

                # g_l[i] = g_u[i-1] = sqrt(a_u[i-1]*a_l[i] + eps)  (cols 0:8)
                # g_u[i]            = sqrt(a_u[i]*a_l[i+1] + eps)  (cols 8:16)
                gq_t = setup.tile([128, 2, NB], F32)
                nc.vector.tensor_mul(gq_t, _ap(a_u[:], 0, [[24, 128], [8, 2], [1, NB]]),
                                     _ap(a_l[:], 8, [[24, 128], [8, 2], [1, NB]]))
                g_t = setup.tile([128, 2, NB], F32)
                nc.scalar.activation(g_t, gq_t, AF.Sqrt, bias=eps_t[:, 0:1])

                # na band values as per-row vectors:
                #   na_b2[:,0,:] = na[i,i-1] = g_l + pr_l*(1-g_l)
                #   na_b2[:,1,:] = na[i,i+1] = g_u + pr_u*(1-g_u)
                omg2 = setup.tile([128, 2, NB], F32)
                nc.vector.tensor_scalar(omg2, g_t, -1.0, 1.0, op0=ALU.mult,
                                        op1=ALU.add)
                prm = setup.tile([128, 2, NB], F32)
                nc.vector.tensor_mul(prm, pr_lu, omg2)
                na_b2 = setup.tile([128, 2, NB], F32)
                nc.vector.tensor_add(na_b2, prm, g_t)

                # u = ln(na[i,i+1] + eps);  nd = na[i,i];  lnnd = ln(nd+eps)
                u_t = setup.tile([128, NB], F32)
                nc.scalar.activation(u_t, na_b2[:, 1, :], AF.Ln,
                                     bias=eps_t[:, 0:1])
                nd_t = setup.tile([128, NB], F32)
                nc.vector.tensor_scalar(nd_t, pr_d, 1.0 - C0, C0,
                                        op0=ALU.mult, op1=ALU.add)
                lnnd = setup.tile([128, NB], F32)
                nc.scalar.activation(lnnd, nd_t, AF.Ln, bias=eps_t[:, 0:1])

                # ---- U = exclusive prefix sum of u (no DRAM round trips) ----
                inc_ps = ps_small.tile([128, NB], F32, tag="tiny")
                nc.tensor.matmul(inc_ps, lhsT=triu_t, rhs=u_t, start=True, stop=True)
                exc_t = setup.tile([128, NB], F32)
                nc.vector.tensor_sub(exc_t, inc_ps, u_t)

                cs_ps = ps_small.tile([1, NB], F32, tag="tiny")   # per-block sums
                nc.tensor.matmul(cs_ps, lhsT=ones_col, rhs=u_t, start=True, stop=True)
                bp_t = setup.tile([1, NB], F32)
                nc.vector.memset(bp_t[:, 0:1], 0.0)
                nc.vector.tensor_copy(bp_t[:, 1:8], cs_ps[0:1, 0:7])
                zer_t = setup.tile([1, NB], F32)
                nc.vector.memset(zer_t, 0.0)
                bpx_t = setup.tile([1, NB], F32)             # exclusive block prefix
                nc.vector.tensor_tensor_scan(bpx_t, bp_t, zer_t, 0.0,
                                             op0=ALU.add, op1=ALU.add)
                bpr_ps = ps_small.tile([128, NB], F32, tag="tiny")
                nc.tensor.matmul(bpr_ps, lhsT=ones_row, rhs=bpx_t, start=True,
                                 stop=True)
                U_t = setup.tile([128, NB], F32)
                nc.vector.tensor_add(U_t, exc_t, bpr_ps)

                # U_rep[p, j] = U[j] via SBUF reshape DMA + ones broadcast matmul
                U_d = dram.tile([S], F32)
                nc.scalar.dma_start(out=_ap(U_d[:], 0, [[1, 128], [128, NB]]),
                                    in_=U_t)
                U_lin = setup.tile([1, S], F32)
                nc.scalar.dma_start(out=U_lin, in_=U_d[:])
                Ur_ps = psrep.tile([128, S], F32, tag="urep")
                for lo in (0, 512):
                    nc.tensor.matmul(Ur_ps[:, lo:lo + 512], lhsT=ones_row,
                                     rhs=U_lin[0:1, lo:lo + 512], start=True,
                                     stop=True)
                Ur_sb = setup.tile([128, S], F32)
                nc.scalar.activation(Ur_sb, Ur_ps, AF.Copy)

            # band3[:, c, r]: c=0 sub-diag na[i,i-1], c=1 diag na[i,i],
            # c=2 super-diag na[i,i+1]; row i = r*128 + p.
            band3 = setup.tile([128, 3, NB], F32)
            nc.vector.tensor_copy(band3[:, 0, :], na_b2[:, 0, :])
            nc.vector.tensor_copy(band3[:, 1, :], nd_t)
            nc.vector.tensor_copy(band3[:, 2, :], na_b2[:, 1, :])

            # ---------------- pass 1: prior loads + na full rows ----------
            pr_ts = []
            for r in range(NB):
                pr_t = prp.tile([128, S], F32, tag="pr")
                nc.sync.dma_start(out=pr_t, in_=prior[r * 128:(r + 1) * 128, :])
                pr_ts.append(pr_t)
            for r in range(NB):
                na_t = napool.tile([128, S], F32, tag="na")
                nc.vector.tensor_scalar(na_t, pr_ts[r], 1.0 - C0, C0,
                                        op0=ALU.mult, op1=ALU.add)
                nc.sync.dma_start(out=na_out[r * 128:(r + 1) * 128, :],
                                  in_=na_t)

            # band overwrite: 3 contiguous elements per row at cols i-1..i+1,
            # partition stride S+1.  Same SP ring as the bulk stores -> FIFO
            # guarantees the band lands after the bulk rows.
            for r in range(NB):
                base = r * 128 * (S + 1) - 1
                if r == 0:
                    # row 0 has no col -1: store cols (0, +1) for p=0,
                    # full 3-wide for p=1..127
                    nc.sync.dma_start(
                        out=_ap(na_out, 0, [[S + 1, 1], [1, 2]]),
                        in_=band3[0:1, 1:3, 0])
                    nc.sync.dma_start(
                        out=_ap(na_out, S, [[S + 1, 127], [1, 3]]),
                        in_=band3[1:128, :, 0])
                elif r == NB - 1:
                    # row 1023 has no col 1024: full 3-wide for p=0..126,
                    # cols (-1, 0) for p=127
                    nc.sync.dma_start(
                        out=_ap(na_out, base, [[S + 1, 127], [1, 3]]),
                        in_=band3[0:127, :, 7])
                    nc.sync.dma_start(
                        out=_ap(na_out, 1023 * (S + 1) - 1, [[S + 1, 1], [1, 2]]),
                        in_=band3[127:128, 0:2, 7])
                else:
                    nc.sync.dma_start(
                        out=_ap(na_out, base, [[S + 1, 128], [1, 3]]),
                        in_=band3[:, :, r])

            # ---------------- pass 2: c_attn rows ----------------
            # cd = |U[j] - U[i]|.  U is non-increasing (u < 0), so left of
            # the diagonal window d >= 0 and right of it d <= 0: one
            # tensor_scalar per region gives |d| directly; a true abs (max
            # of +/-d) is only needed in the 130-wide diagonal window.
            # The diag is pre-patched so exp(-cd) lands na[i,i] there.
            for r in range(NB):
                w0 = r * 128 - 1
                wlo = max(w0, 0)
                whi = min(w0 + 130, S)
                wd = whi - wlo
                mo = wlo - w0
                Ui = U_t[:, r:r + 1]

                cd_t = cdpool.tile([128, S], F32, tag="cd")
                if wlo > 0:
                    nc.vector.tensor_scalar(cd_t[:, 0:wlo], Ur_sb[:, 0:wlo],
                                            Ui, None, op0=ALU.subtract)
                if whi < S:
                    nc.vector.tensor_scalar(cd_t[:, whi:S], Ur_sb[:, whi:S],
                                            Ui, -1.0, op0=ALU.subtract,
                                            op1=ALU.mult)
                ta_t = cdpool.tile([128, 130], F32, tag="ta")
                nc.vector.tensor_scalar(ta_t[:, :wd], Ur_sb[:, wlo:whi],
                                        Ui, None, op0=ALU.subtract)
                tb_t = cdpool.tile([128, 130], F32, tag="tb")
                nc.vector.tensor_scalar(tb_t[:, :wd], Ur_sb[:, wlo:whi],
                                        Ui, -1.0, op0=ALU.subtract,
                                        op1=ALU.mult)
                t5w = cdpool.tile([128, 130], F32, tag="t5w")
                nc.vector.tensor_scalar(t5w[:, :wd], md_t[:, mo:mo + wd],
                                        lnnd[:, r:r + 1], None, op0=ALU.mult)
                nc.vector.tensor_max(cd_t[:, wlo:whi], ta_t[:, :wd],
                                     tb_t[:, :wd])
                nc.vector.tensor_sub(cd_t[:, wlo:whi], cd_t[:, wlo:whi],
                                     t5w[:, :wd])
                c2_t = c2pool.tile([128, S], F32, tag="c2")
                nc.scalar.activation(c2_t, cd_t, AF.Exp, scale=-1.0)
                nc.scalar.dma_start(out=c_out[r * 128:(r + 1) * 128, :],
                                    in_=c2_t)

    _split_multi_waits(nc)
    return nc


def _get_nc():
    if "nc" not in _CACHE:
        _CACHE["nc"] = build_nc()
    return _CACHE["nc"]


def run(inputs, trace=False, tmpdir=None):
    nc = _get_nc()
    context = np.asarray(inputs["context"], np.float32)
    prior = np.asarray(inputs["prior"], np.float32)
    w = np.asarray(inputs["proj_weight"], np.float32)
    bias = np.asarray(inputs["proj_bias"], np.float32)

    wT = np.ascontiguousarray(w.T)                     # [E, 2P]
    bcol = np.ascontiguousarray(bias.reshape(P2, 1))
    in_maps = []
    for b in range(B):
        in_maps.append({
            "xT": np.ascontiguousarray(context[:, b, :].T),   # [E, S]
            "wT": wT,
            "bvec": bcol,
            "prior": np.ascontiguousarray(prior[b]),
        })
    try:
        res = run_bass_kernel_spmd(nc, in_maps, list(range(B)), trace=trace,
                                   tmpdir=tmpdir)
    except ModuleNotFoundError:
        res = run_bass_kernel_spmd(nc, in_maps, list(range(B)), trace=False)
    c = np.stack([res.results[i]["c_out"] for i in range(B)])
    na = np.stack([res.results[i]["na_out"] for i in range(B)])
    return (c, na), res


def kernel(**inputs):
    (c, na), _ = run(inputs)
    return (c, na)


# revision 14
# speedup vs baseline: 1.9919x; 1.0665x over previous
"""ConstituentAttention Trainium2 kernel.

Math (derived from the reference):
  - score is masked to the super/sub-diagonal only, so the row softmax is a
    2-element softmax: a_u[i] = sigmoid((s_u[i]-s_l[i])/E), a_l = 1-a_u,
    where s_u[i] = q_i.k_{i+1}, s_l[i] = q_i.k_{i-1}.
  - neighbor_attn = prior + (1-prior)*g where g == sqrt(1e-9) =: C0 everywhere
    except g[i,i+1] = g[i+1,i] = sqrt(a_u[i]*a_l[i+1] + 1e-9) =: g_u[i].
  - log-space prefix products collapse to c_attn[i,j] = exp(-|U[j]-U[i]|) for
    i != j, where U = exclusive prefix sum of u_i = log(na[i,i+1] + 1e-9);
    diagonal of c_attn = na[i,i].

Sharding: data-parallel over batch, one batch element per NeuronCore (B=8).

Engine discipline (the v1 lesson): DVE 2-port ops and GpSimd take an
exclusive lock on the shared SBUF port pair - concurrent DVE+GpSimd work
stretches BOTH by ~15x.  So: all elementwise compute lives on Vector, all
activations on Scalar, GpSimd only issues the early scatter-gather DMAs.

Band handling: the tri-diagonal na values are computed as tiny [128, NB]
vectors and written with narrow 3-elements-per-row strided stores AFTER the
bulk rows (same HWDGE ring -> FIFO gives WAW order).  The c diagonal is
patched pre-exp in SBUF: cd[i,i] -= ln(na[i,i]) so exp(-cd) lands na[i,i].
"""

import numpy as np

import concourse.bass as bass
import concourse.tile as tile
from concourse import mybir
from concourse.bass_utils import run_bass_kernel_spmd

S, B, E, P = 1024, 8, 512, 64
P2 = 2 * P
NB = S // 128
C0 = float(np.sqrt(1e-9))
NEG = -1e30
F32 = mybir.dt.float32
F32R = mybir.dt.float32r
BF16 = mybir.dt.bfloat16
AF = mybir.ActivationFunctionType
ALU = mybir.AluOpType

_CACHE = {}


def _ap(handle_or_ap, offset, dims):
    a0 = handle_or_ap[:] if not isinstance(handle_or_ap, bass.AP) else handle_or_ap
    return bass.AP(tensor=a0.tensor, offset=offset, ap=[list(d) for d in dims])


def _split_multi_waits(nc):
    """This toolchain's walrus accepts at most ONE embedded on_wait per
    instruction; hoist extras into standalone EventSemaphore waits just
    before the instruction on the same engine."""
    n = 0
    for bb in nc.main_func.blocks:
        new = []
        for ins in bb.instructions:
            si = ins.sync_info
            if si is not None and si.on_wait and len(si.on_wait) > 1:
                for w in si.on_wait[:-1]:
                    n += 1
                    wi = mybir.InstEventSemaphore(
                        name=f"I-waitsplit-{n}",
                        opcode="EventSemaphore",
                        engine=ins.engine,
                        sync_info=mybir.SyncInfo(on_wait=[w], on_update=[]),
                    )
                    try:
                        nc.register_instruction(wi)
                    except Exception:
                        pass
                    new.append(wi)
                si.on_wait = si.on_wait[-1:]
            new.append(ins)
        try:
            bb.instructions[:] = new
        except TypeError:
            bb.instructions = new
    return n


def build_nc():
    nc = bass.Bass()

    xT = nc.dram_tensor("xT", [E, S], F32, kind="ExternalInput")
    wT = nc.dram_tensor("wT", [E, P2], F32, kind="ExternalInput")
    bvec = nc.dram_tensor("bvec", [P2, 1], F32, kind="ExternalInput")
    prior = nc.dram_tensor("prior", [S, S], F32, kind="ExternalInput")
    na_out = nc.dram_tensor("na_out", [S, S], F32, kind="ExternalOutput")
    c_out = nc.dram_tensor("c_out", [S, S], F32, kind="ExternalOutput")

    # diag mask [128,130]: for row-block r the band lives in absolute cols
    # [r*128-1, r*128+129); with window origin w0 = r*128-1 the diagonal sits
    # at rel col p+1, independent of r.
    p_i = np.arange(128)[:, None]
    c_i = np.arange(130)[None, :]
    md_h = nc.inline_tensor((c_i == p_i + 1).astype(np.float32), "mask_d")
    mu_h = nc.inline_tensor((c_i == p_i + 2).astype(np.float32), "mask_u")
    ml_h = nc.inline_tensor((c_i == p_i).astype(np.float32), "mask_l")
    # lhsT for within-block inclusive cumsum over partitions: out = triu.T @ u
    triu_h = nc.inline_tensor(
        np.triu(np.ones((128, 128), np.float32)), "triu_ones"
    )
    ones_col_h = nc.inline_tensor(np.ones((128, 1), np.float32), "ones_col")
    ones_row_h = nc.inline_tensor(np.ones((1, 128), np.float32), "ones_row")

    with tile.TileContext(nc) as tc:
        with (
            tc.tile_pool(name="setup", bufs=1) as setup,
            tc.tile_pool(name="na", bufs=4) as napool,
            tc.tile_pool(name="cdp", bufs=3) as cdpool,
            tc.tile_pool(name="c2p", bufs=3) as c2pool,
            tc.tile_pool(name="prp", bufs=8) as prp,
            tc.tile_pool(name="mm", bufs=2, space="PSUM") as mm,
            tc.tile_pool(name="mm1", bufs=2, space="PSUM") as mm1,
            tc.tile_pool(name="ps_small", bufs=2, space="PSUM") as ps_small,
            tc.tile_pool(name="psrep", bufs=1, space="PSUM") as psrep,
            tc.tile_pool(name="dram", bufs=1, space="DRAM") as dram,
        ):
            # ------ critical-path loads first: xT/wT as bf16 cast-on-DMA ---
            # SWDGE (gpsimd) casts f32->bf16 in flight; HBM read bytes are
            # unchanged but the qk matmuls run at bf16 rate.  The scores are
            # divided by E=512 and flattened through a sigmoid, so bf16 noise
            # lands ~2e-3 on the final c_attn - well inside the 2e-2 gate.
            xT_t = setup.tile([128, 4, S], BF16)
            wT_t = setup.tile([128, 4, P2], BF16)
            bias_t = setup.tile([128, 1], F32)
            for c in range(4):
                nc.gpsimd.dma_start(
                    out=xT_t[:, c, :],
                    in_=_ap(xT, c * 128 * S, [[S, 128], [1, S]]))
            nc.gpsimd.dma_start(
                out=wT_t,
                in_=_ap(wT, 0, [[P2, 128], [128 * P2, 4], [1, P2]]))
            nc.sync.dma_start(out=bias_t, in_=bvec[:])

            # ------- prior band gathers (early; feed U chain + band3) ------
            # pr_lu[:, 0, :] = prior[i, i-1] (row 0 unused -> 0)
            # pr_lu[:, 1, :] = prior[i, i+1] (row 1023 unused -> 0)
            pr_lu = setup.tile([128, 2, NB], F32)
            nc.vector.memset(pr_lu[0:1, 0, 0:1], 0.0)
            nc.vector.memset(pr_lu[:, 1, 7:8], 0.0)
            nc.gpsimd.dma_start(
                out=pr_lu[1:128, 0, 0:1],
                in_=_ap(prior, S, [[S + 1, 127], [1, 1]]))
            nc.gpsimd.dma_start(
                out=pr_lu[:, 0, 1:8],
                in_=_ap(prior, 128 * (S + 1) - 1,
                        [[S + 1, 128], [128 * (S + 1), 7]]))
            nc.gpsimd.dma_start(
                out=pr_lu[:, 1, 0:7],
                in_=_ap(prior, 1, [[S + 1, 128], [128 * (S + 1), 7]]))
            nc.gpsimd.dma_start(
                out=pr_lu[0:127, 1, 7:8],
                in_=_ap(prior, 896 * (S + 1) + 1, [[S + 1, 127], [1, 1]]))
            pr_d = setup.tile([128, NB], F32)            # prior[i, i]
            nc.gpsimd.dma_start(
                out=pr_d, in_=_ap(prior, 0, [[S + 1, 128], [128 * (S + 1), 8]]))

            # constants via the Act queue (fast HWDGE; Act idle this early)
            md_t = setup.tile([128, 130], F32)
            nc.scalar.dma_start(out=md_t, in_=md_h[:])
            mu_t = setup.tile([128, 130], F32)
            nc.scalar.dma_start(out=mu_t, in_=mu_h[:])
            ml_t = setup.tile([128, 130], F32)
            nc.scalar.dma_start(out=ml_t, in_=ml_h[:])
            triu_t = setup.tile([128, 128], F32)
            nc.scalar.dma_start(out=triu_t, in_=triu_h[:])
            ones_col = setup.tile([128, 1], F32)
            nc.scalar.dma_start(out=ones_col, in_=ones_col_h[:])
            ones_row = setup.tile([1, 128], F32)
            nc.scalar.dma_start(out=ones_row, in_=ones_row_h[:])

            # preload the Sigmoid activation table during the idle head
            eps_t = setup.tile([128, 1], F32)
            nc.vector.memset(eps_t, 1e-9)
            warm_t = setup.tile([1, 1], F32)
            nc.scalar.activation(warm_t, eps_t[0:1, 0:1], AF.Sigmoid)

            # PE clock warmup: dummy matmuls (results are garbage; the Ur
            # broadcast later overwrites this PSUM bank).
            warm_ps = psrep.tile([128, S], F32, tag="urep")
            for d in range(8):
                nc.tensor.matmul(warm_ps[0:64, 0:128],
                                 lhsT=wT_t[:, 0, 0:64],
                                 rhs=wT_t[:, 0, :],
                                 start=True, stop=True)

            # ---------------- qT/kT = (x @ W.T).T halves  [64, S] ----------
            # fp32r matmuls: out free 512 >= 256 -> 1 cycle/row.  The product
            # chain is split at col 511 and interleaved with the j-halves so
            # band extraction for cols [0,511) overlaps the j=1 matmuls.
            qT_t = setup.tile([64, S], F32)
            kT_t = setup.tile([64, S], F32)
            su_st = setup.tile([1, 1026], F32)
            sl_st = setup.tile([1, 1026], F32)
            tu_t = setup.tile([64, S - 1], F32)
            tl_t = setup.tile([64, S - 1], F32)
            with tc.high_priority():
                nc.vector.memset(su_st[:, 0:1], NEG)
                nc.vector.memset(su_st[:, 1024:1026], NEG)
                nc.vector.memset(sl_st[:, 0:2], NEG)
                nc.vector.memset(sl_st[:, 1025:1026], NEG)
                for j in range(2):
                    for half, dest_t in enumerate((qT_t, kT_t)):
                        ps = mm.tile([64, 512], F32, tag="mmbig")
                        for c in range(4):
                            nc.tensor.matmul(
                                ps[:],
                                lhsT=wT_t[:, c, half * 64:(half + 1) * 64],
                                rhs=xT_t[:, c, j * 512:(j + 1) * 512],
                                start=(c == 0),
                                stop=(c == 3),
                            )
                        # bias adds on DVE only: keeps the ACT table sequence
                        # at Sigmoid->Sqrt->Ln->Exp (4 loads, no thrash)
                        nc.vector.tensor_scalar_add(
                            dest_t[:, j * 512:(j + 1) * 512], ps,
                            bias_t[half * 64:(half + 1) * 64, 0:1])
                    # band products for the cols this j-half completes:
                    #   su_stage[k] = s_u[k-1] = tu[k-1]  (s_u[1023] = -inf)
                    #   sl_stage[k] = s_l[k-1] = tl[k-2]  (s_l[0] = -inf)
                    lo, hi = (0, 511) if j == 0 else (511, 1023)
                    w = hi - lo
                    nc.vector.tensor_mul(tu_t[:, lo:hi], qT_t[:, lo:hi],
                                         kT_t[:, lo + 1:hi + 1])
                    nc.vector.tensor_mul(tl_t[:, lo:hi], qT_t[:, lo + 1:hi + 1],
                                         kT_t[:, lo:hi])
                    for src_t, st_t, off in ((tu_t, su_st, 1),
                                             (tl_t, sl_st, 2)):
                        ps1 = mm1.tile([1, 512], F32, tag="ones")
                        nc.tensor.matmul(ps1[0:1, 0:w],
                                         lhsT=ones_col[0:64, :],
                                         rhs=src_t[:, lo:hi],
                                         start=True, stop=True)
                        nc.vector.tensor_copy(st_t[:, off + lo:off + hi],
                                              ps1[0:1, 0:w])

            # [128, 24] stacks: col groups g=0,1,2 hold offsets i-1,i,i+1
            with tc.high_priority():
                su_d = dram.tile([1026], F32)
                nc.scalar.dma_start(out=su_d[:], in_=su_st)
                sl_d = dram.tile([1026], F32)
                nc.scalar.dma_start(out=sl_d[:], in_=sl_st)
                s_uu = setup.tile([128, 3, NB], F32)
                s_ll = setup.tile([128, 3, NB], F32)
                for g in range(3):
                    nc.scalar.dma_start(
                        out=s_uu[:, g, :],
                        in_=_ap(su_d[:], g, [[1, 128], [128, NB]]))
                    nc.scalar.dma_start(
                        out=s_ll[:, g, :],
                        in_=_ap(sl_d[:], g, [[1, 128], [128, NB]]))

                # 2-element softmax via sigmoid on all 3 offset groups at once
                diff_t = setup.tile([128, 3, NB], F32)
                nc.vector.tensor_sub(diff_t, s_uu, s_ll)
                a_u = setup.tile([128, 3, NB], F32)
                nc.scalar.activation(a_u, diff_t, AF.Sigmoid, scale=1.0 / E)
                a_l = setup.tile([128, 3, NB], F32)
                nc.scalar.activation(a_l, diff_t, AF.Sigmoid, scale=-1.0 / E)

                # g_l[i] = g_u[i-1] = sqrt(a_u[i-1]*a_l[i] + eps)  (cols 0:8)
                # g_u[i]            = sqrt(a_u[i]*a_l[i+1] + eps)  (cols 8:16)
                gq_t = setup.tile([128, 2, NB], F32)
                nc.vector.tensor_mul(gq_t, _ap(a_u[:], 0, [[24, 128], [8, 2], [1, NB]]),
                                     _ap(a_l[:], 8, [[24, 128], [8, 2], [1, NB]]))
                g_t = setup.tile([128, 2, NB], F32)
                nc.scalar.activation(g_t, gq_t, AF.Sqrt, bias=eps_t[:, 0:1])

                # na band values as per-row vectors:
                #   na_b2[:,0,:] = na[i,i-1] = g_l + pr_l*(1-g_l)
                #   na_b2[:,1,:] = na[i,i+1] = g_u + pr_u*(1-g_u)
                omg2 = setup.tile([128, 2, NB], F32)
                nc.vector.tensor_scalar(omg2, g_t, -1.0, 1.0, op0=ALU.mult,
                                        op1=ALU.add)
                prm = setup.tile([128, 2, NB], F32)
                nc.vector.tensor_mul(prm, pr_lu, omg2)
                na_b2 = setup.tile([128, 2, NB], F32)
                nc.vector.tensor_add(na_b2, prm, g_t)

                # u = ln(na[i,i+1] + eps);  nd = na[i,i];  lnnd = ln(nd+eps)
                u_t = setup.tile([128, NB], F32)
                nc.scalar.activation(u_t, na_b2[:, 1, :], AF.Ln,
                                     bias=eps_t[:, 0:1])
                nd_t = setup.tile([128, NB], F32)
                nc.vector.tensor_scalar(nd_t, pr_d, 1.0 - C0, C0,
                                        op0=ALU.mult, op1=ALU.add)
                lnnd = setup.tile([128, NB], F32)
                nc.scalar.activation(lnnd, nd_t, AF.Ln, bias=eps_t[:, 0:1])

                # band-correction scalars vs the bulk na formula:
                #   na_band - na_bulk = (g - C0) * (1 - prior)  at [i, i-/+1]
                pu1_2 = setup.tile([128, 2, NB], F32)
                nc.vector.tensor_scalar(pu1_2, pr_lu, -1.0, 1.0, op0=ALU.mult,
                                        op1=ALU.add)
                gc_2 = setup.tile([128, 2, NB], F32)
                nc.vector.tensor_scalar(gc_2, g_t, C0, None, op0=ALU.subtract)
                cu_2 = setup.tile([128, 2, NB], F32)
                nc.vector.tensor_mul(cu_2, gc_2, pu1_2)

                # ---- U = exclusive prefix sum of u (no DRAM round trips) ----
                inc_ps = ps_small.tile([128, NB], F32, tag="tiny")
                nc.tensor.matmul(inc_ps, lhsT=triu_t, rhs=u_t, start=True, stop=True)
                exc_t = setup.tile([128, NB], F32)
                nc.vector.tensor_sub(exc_t, inc_ps, u_t)

                cs_ps = ps_small.tile([1, NB], F32, tag="tiny")   # per-block sums
                nc.tensor.matmul(cs_ps, lhsT=ones_col, rhs=u_t, start=True, stop=True)
                bp_t = setup.tile([1, NB], F32)
                nc.vector.memset(bp_t[:, 0:1], 0.0)
                nc.vector.tensor_copy(bp_t[:, 1:8], cs_ps[0:1, 0:7])
                zer_t = setup.tile([1, NB], F32)
                nc.vector.memset(zer_t, 0.0)
                bpx_t = setup.tile([1, NB], F32)             # exclusive block prefix
                nc.vector.tensor_tensor_scan(bpx_t, bp_t, zer_t, 0.0,
                                             op0=ALU.add, op1=ALU.add)
                bpr_ps = ps_small.tile([128, NB], F32, tag="tiny")
                nc.tensor.matmul(bpr_ps, lhsT=ones_row, rhs=bpx_t, start=True,
                                 stop=True)
                U_t = setup.tile([128, NB], F32)
                nc.vector.tensor_add(U_t, exc_t, bpr_ps)

                # U_rep[p, j] = U[j] via SBUF reshape DMA + ones broadcast matmul
                U_d = dram.tile([S], F32)
                nc.scalar.dma_start(out=_ap(U_d[:], 0, [[1, 128], [128, NB]]),
                                    in_=U_t)
                U_lin = setup.tile([1, S], F32)
                nc.scalar.dma_start(out=U_lin, in_=U_d[:])
                Ur_ps = psrep.tile([128, S], F32, tag="urep")
                for lo in (0, 512):
                    nc.tensor.matmul(Ur_ps[:, lo:lo + 512], lhsT=ones_row,
                                     rhs=U_lin[0:1, lo:lo + 512], start=True,
                                     stop=True)

            # ---------------- pass 1: prior loads + na full rows ----------
            pr_ts = []
            for r in range(NB):
                pr_t = prp.tile([128, S], F32, tag="pr")
                nc.sync.dma_start(out=pr_t, in_=prior[r * 128:(r + 1) * 128, :])
                pr_ts.append(pr_t)
            for r in range(NB):
                na_t = napool.tile([128, S], F32, tag="na")
                nc.vector.tensor_scalar(na_t, pr_ts[r], 1.0 - C0, C0,
                                        op0=ALU.mult, op1=ALU.add)
                nc.sync.dma_start(out=na_out[r * 128:(r + 1) * 128, :],
                                  in_=na_t)

            # band overwrite: rebuild the [128,130] window from pr_t with the
            # two off-diagonal corrections added, stored as 130-wide rows
            # (520 B/descriptor; 4 B-wide diag scatters grind the SDMA
            # engines with read-modify-writes).  Same SP ring as the bulk
            # stores -> FIFO gives WAW order.
            for r in range(NB):
                w0 = r * 128 - 1
                wlo = max(w0, 0)
                whi = min(w0 + 130, S)
                wd = whi - wlo
                mo = wlo - w0
                bw_t = napool.tile([128, 130], F32, tag="bw")
                nc.vector.tensor_scalar(bw_t[:, :wd], pr_ts[r][:, wlo:whi],
                                        1.0 - C0, C0, op0=ALU.mult, op1=ALU.add)
                t1w = napool.tile([128, 130], F32, tag="t1w")
                nc.vector.tensor_scalar(t1w[:, :wd], mu_t[:, mo:mo + wd],
                                        cu_2[:, 1, r:r + 1], None, op0=ALU.mult)
                nc.vector.tensor_add(bw_t[:, :wd], bw_t[:, :wd], t1w[:, :wd])
                t2w = napool.tile([128, 130], F32, tag="t2w")
                nc.vector.tensor_scalar(t2w[:, :wd], ml_t[:, mo:mo + wd],
                                        cu_2[:, 0, r:r + 1], None, op0=ALU.mult)
                nc.vector.tensor_add(bw_t[:, :wd], bw_t[:, :wd], t2w[:, :wd])
                nc.sync.dma_start(
                    out=_ap(na_out, r * 128 * S + wlo, [[S, 128], [1, wd]]),
                    in_=bw_t[:, :wd])

            # ---------------- pass 2: c_attn rows ----------------
            # cd = |U[j] - U[i]|.  U is non-increasing (u < 0), so left of
            # the diagonal window d >= 0 and right of it d <= 0: one
            # tensor_scalar per region gives |d| directly; a true abs (max
            # of +/-d) is only needed in the 130-wide diagonal window.
            # The diag is pre-patched so exp(-cd) lands na[i,i] there.
            for r in range(NB):
                w0 = r * 128 - 1
                wlo = max(w0, 0)
                whi = min(w0 + 130, S)
                wd = whi - wlo
                mo = wlo - w0
                Ui = U_t[:, r:r + 1]

                cd_t = cdpool.tile([128, S], F32, tag="cd")
                if wlo > 0:
                    nc.vector.tensor_scalar(cd_t[:, 0:wlo], Ur_ps[:, 0:wlo],
                                            Ui, None, op0=ALU.subtract)
                if whi < S:
                    nc.vector.tensor_scalar(cd_t[:, whi:S], Ur_ps[:, whi:S],
                                            Ui, -1.0, op0=ALU.subtract,
                                            op1=ALU.mult)
                ta_t = cdpool.tile([128, 130], F32, tag="ta")
                nc.vector.tensor_scalar(ta_t[:, :wd], Ur_ps[:, wlo:whi],
                                        Ui, None, op0=ALU.subtract)
                tb_t = cdpool.tile([128, 130], F32, tag="tb")
                nc.vector.tensor_scalar(tb_t[:, :wd], Ur_ps[:, wlo:whi],
                                        Ui, -1.0, op0=ALU.subtract,
                                        op1=ALU.mult)
                t5w = cdpool.tile([128, 130], F32, tag="t5w")
                nc.vector.tensor_scalar(t5w[:, :wd], md_t[:, mo:mo + wd],
                                        lnnd[:, r:r + 1], None, op0=ALU.mult)
                nc.vector.tensor_max(cd_t[:, wlo:whi], ta_t[:, :wd],
                                     tb_t[:, :wd])
                nc.vector.tensor_sub(cd_t[:, wlo:whi], cd_t[:, wlo:whi],
                                     t5w[:, :wd])
                c2_t = c2pool.tile([128, S], F32, tag="c2")
                nc.scalar.activation(c2_t, cd_t, AF.Exp, scale=-1.0)
                nc.scalar.dma_start(out=c_out[r * 128:(r + 1) * 128, :],
                                    in_=c2_t)

    _split_multi_waits(nc)
    return nc


def _get_nc():
    if "nc" not in _CACHE:
        _CACHE["nc"] = build_nc()
    return _CACHE["nc"]


def run(inputs, trace=False, tmpdir=None):
    nc = _get_nc()
    context = np.asarray(inputs["context"], np.float32)
    prior = np.asarray(inputs["prior"], np.float32)
    w = np.asarray(inputs["proj_weight"], np.float32)
    bias = np.asarray(inputs["proj_bias"], np.float32)

    wT = np.ascontiguousarray(w.T)                     # [E, 2P]
    bcol = np.ascontiguousarray(bias.reshape(P2, 1))
    in_maps = []
    for b in range(B):
        in_maps.append({
            "xT": np.ascontiguousarray(context[:, b, :].T),   # [E, S]
            "wT": wT,
            "bvec": bcol,
            "prior": np.ascontiguousarray(prior[b]),
        })
    try:
        res = run_bass_kernel_spmd(nc, in_maps, list(range(B)), trace=trace,
                                   tmpdir=tmpdir)
    except ModuleNotFoundError:
        res = run_bass_kernel_spmd(nc, in_maps, list(range(B)), trace=False)
    c = np.stack([res.results[i]["c_out"] for i in range(B)])
    na = np.stack([res.results[i]["na_out"] for i in range(B)])
    return (c, na), res


def kernel(**inputs):
    (c, na), _ = run(inputs)
    return (c, na)


# revision 20
# speedup vs baseline: 2.3719x; 1.1907x over previous
"""ConstituentAttention Trainium2 kernel.

Math (derived from the reference):
  - score is masked to the super/sub-diagonal only, so the row softmax is a
    2-element softmax: a_u[i] = sigmoid((s_u[i]-s_l[i])/E), a_l = 1-a_u,
    where s_u[i] = q_i.k_{i+1}, s_l[i] = q_i.k_{i-1}.
  - neighbor_attn = prior + (1-prior)*g where g == sqrt(1e-9) =: C0 everywhere
    except g[i,i+1] = g[i+1,i] = sqrt(a_u[i]*a_l[i+1] + 1e-9) =: g_u[i].
  - log-space prefix products collapse to c_attn[i,j] = exp(-|U[j]-U[i]|) for
    i != j, where U = exclusive prefix sum of u_i = log(na[i,i+1] + 1e-9);
    diagonal of c_attn = na[i,i].

Sharding: data-parallel over batch, one batch element per NeuronCore (B=8).

Engine discipline (v1 lesson): DVE 2-port ops and GpSimd compute take an
exclusive lock on the shared SBUF port pair - concurrent DVE+GpSimd work
stretches BOTH ~15x.  All elementwise compute lives on Vector, activations
on Scalar, GpSimd only issues the early scatter-gather DMAs.

Data-movement discipline (v2/v3 lessons):
  - diag-scatter stores (4-12 B descriptors) grind the SDMA engines with HBM
    read-modify-writes; the band overwrite uses 130-wide row strips instead.
  - DRAM staging round trips for cross-partition reshapes stall 10-20 us
    behind bulk traffic.  ALL reshapes now ride the PE array: the banded
    score differences s_u[i+d]-s_l[i+d] land directly in [128, NB] layout
    via 48 tiny matmuls (lhsT = shifted 128-col slices of the product rows,
    rhs = +/-ones accumulating in PSUM), and U[128,NB] -> row layout goes
    through identity-matmul transposes.  Zero staging DMAs.
  - bf16 cast on DVE for the qk matmuls (scores are /E then sigmoided; bf16
    noise lands ~1e-4 on the outputs, the gate is 2e-2).
"""

import numpy as np

import concourse.bass as bass
import concourse.tile as tile
from concourse import mybir
from concourse.bass_utils import run_bass_kernel_spmd

S, B, E, P = 1024, 8, 512, 64
P2 = 2 * P
NB = S // 128
C0 = float(np.sqrt(1e-9))
F32 = mybir.dt.float32
BF16 = mybir.dt.bfloat16
AF = mybir.ActivationFunctionType
ALU = mybir.AluOpType

_CACHE = {}


def _ap(handle_or_ap, offset, dims):
    a0 = handle_or_ap[:] if not isinstance(handle_or_ap, bass.AP) else handle_or_ap
    return bass.AP(tensor=a0.tensor, offset=offset, ap=[list(d) for d in dims])


def _split_multi_waits(nc):
    """This toolchain's walrus accepts at most ONE embedded on_wait per
    instruction; hoist extras into standalone EventSemaphore waits just
    before the instruction on the same engine."""
    n = 0
    for bb in nc.main_func.blocks:
        new = []
        for ins in bb.instructions:
            si = ins.sync_info
            if si is not None and si.on_wait and len(si.on_wait) > 1:
                for w in si.on_wait[:-1]:
                    n += 1
                    wi = mybir.InstEventSemaphore(
                        name=f"I-waitsplit-{n}",
                        opcode="EventSemaphore",
                        engine=ins.engine,
                        sync_info=mybir.SyncInfo(on_wait=[w], on_update=[]),
                    )
                    try:
                        nc.register_instruction(wi)
                    except Exception:
                        pass
                    new.append(wi)
                si.on_wait = si.on_wait[-1:]
            new.append(ins)
        try:
            bb.instructions[:] = new
        except TypeError:
            bb.instructions = new
    return n


def build_nc():
    nc = bass.Bass()

    xT = nc.dram_tensor("xT", [E, S], F32, kind="ExternalInput")
    wT = nc.dram_tensor("wT", [E, P2], F32, kind="ExternalInput")
    bvec = nc.dram_tensor("bvec", [P2, 1], F32, kind="ExternalInput")
    prior = nc.dram_tensor("prior", [S, S], F32, kind="ExternalInput")
    na_out = nc.dram_tensor("na_out", [S, S], F32, kind="ExternalOutput")
    c_out = nc.dram_tensor("c_out", [S, S], F32, kind="ExternalOutput")

    # masks [128,130]: for row-block r the band lives in absolute cols
    # [r*128-1, r*128+129); with window origin w0 = r*128-1 the diag sits at
    # rel col p+1, super at p+2, sub at p, independent of r.
    p_i = np.arange(128)[:, None]
    c_i = np.arange(130)[None, :]
    md_h = nc.inline_tensor((c_i == p_i + 1).astype(np.float32), "mask_d")
    mu_h = nc.inline_tensor((c_i == p_i + 2).astype(np.float32), "mask_u")
    ml_h = nc.inline_tensor((c_i == p_i).astype(np.float32), "mask_l")
    # lhsT for within-block inclusive cumsum over partitions: out = triu.T @ u
    triu_h = nc.inline_tensor(
        np.triu(np.ones((128, 128), np.float32)), "triu_ones"
    )
    ident_h = nc.inline_tensor(np.eye(128, dtype=np.float32), "ident")
    ones_col_h = nc.inline_tensor(np.ones((128, 1), np.float32), "ones_col")
    nones_col_h = nc.inline_tensor(np.full((128, 1), -1.0, np.float32),
                                   "nones_col")
    ones_row_h = nc.inline_tensor(np.ones((1, 128), np.float32), "ones_row")

    with tile.TileContext(nc) as tc:
        with (
            tc.tile_pool(name="setup", bufs=1) as setup,
            tc.tile_pool(name="na", bufs=4) as napool,
            tc.tile_pool(name="cdp", bufs=3) as cdpool,
            tc.tile_pool(name="c2p", bufs=3) as c2pool,
            tc.tile_pool(name="prp", bufs=8) as prp,
            tc.tile_pool(name="mm", bufs=2, space="PSUM") as mm,
            tc.tile_pool(name="ps_small", bufs=1, space="PSUM") as ps_small,
            tc.tile_pool(name="psu", bufs=1, space="PSUM") as psu,
            tc.tile_pool(name="psd", bufs=1, space="PSUM") as psd,
            tc.tile_pool(name="psrep", bufs=1, space="PSUM") as psrep,
        ):
            # -------- critical-path loads first on SP: xT chunks + wT -------
            xT_t = setup.tile([128, 4, S], F32)
            wT_t = setup.tile([128, 4, P2], F32)
            bias_t = setup.tile([128, 1], F32)
            nc.sync.dma_start(
                out=xT_t[:, 0, :], in_=_ap(xT, 0, [[S, 128], [1, S]]))
            nc.sync.dma_start(
                out=wT_t,
                in_=_ap(wT, 0, [[P2, 128], [128 * P2, 4], [1, P2]]))
            nc.sync.dma_start(out=bias_t, in_=bvec[:])
            for c in range(1, 4):
                nc.sync.dma_start(
                    out=xT_t[:, c, :],
                    in_=_ap(xT, c * 128 * S, [[S, 128], [1, S]]))

            # ------- prior band gathers (gpsimd SWDGE; queue is idle) ------
            # pr_lu[:, 0, :] = prior[i, i-1] (row 0 unused -> 0)
            # pr_lu[:, 1, :] = prior[i, i+1] (row 1023 unused -> 0)
            pr_lu = setup.tile([128, 2, NB], F32)
            nc.vector.memset(pr_lu[0:1, 0, 0:1], 0.0)
            nc.vector.memset(pr_lu[:, 1, 7:8], 0.0)
            nc.gpsimd.dma_start(
                out=pr_lu[1:128, 0, 0:1],
                in_=_ap(prior, S, [[S + 1, 127], [1, 1]]))
            nc.gpsimd.dma_start(
                out=pr_lu[:, 0, 1:8],
                in_=_ap(prior, 128 * (S + 1) - 1,
                        [[S + 1, 128], [128 * (S + 1), 7]]))
            nc.gpsimd.dma_start(
                out=pr_lu[:, 1, 0:7],
                in_=_ap(prior, 1, [[S + 1, 128], [128 * (S + 1), 7]]))
            nc.gpsimd.dma_start(
                out=pr_lu[0:127, 1, 7:8],
                in_=_ap(prior, 896 * (S + 1) + 1, [[S + 1, 127], [1, 1]]))
            pr_d = setup.tile([128, NB], F32)            # prior[i, i]
            nc.gpsimd.dma_start(
                out=pr_d, in_=_ap(prior, 0, [[S + 1, 128], [128 * (S + 1), 8]]))

            # constants via the Act queue
            md_t = setup.tile([128, 130], F32)
            nc.scalar.dma_start(out=md_t, in_=md_h[:])
            mu_t = setup.tile([128, 130], F32)
            nc.scalar.dma_start(out=mu_t, in_=mu_h[:])
            ml_t = setup.tile([128, 130], F32)
            nc.scalar.dma_start(out=ml_t, in_=ml_h[:])
            triu_t = setup.tile([128, 128], F32)
            nc.scalar.dma_start(out=triu_t, in_=triu_h[:])
            ident_t = setup.tile([128, 128], F32)
            nc.scalar.dma_start(out=ident_t, in_=ident_h[:])
            ones_col = setup.tile([128, 1], F32)
            nc.scalar.dma_start(out=ones_col, in_=ones_col_h[:])
            nones_col = setup.tile([128, 1], F32)
            nc.scalar.dma_start(out=nones_col, in_=nones_col_h[:])
            ones_row = setup.tile([1, 128], F32)
            nc.scalar.dma_start(out=ones_row, in_=ones_row_h[:])

            # preload the Sigmoid activation table during the idle head
            eps_t = setup.tile([128, 1], F32)
            nc.vector.memset(eps_t, 1e-9)
            warm_t = setup.tile([1, 1], F32)
            nc.scalar.activation(warm_t, eps_t[0:1, 0:1], AF.Sigmoid)

            # bf16 copies of x/W for the qk matmuls (DVE is idle this early)
            xb_t = setup.tile([128, 4, S], BF16)
            wb_t = setup.tile([128, 4, P2], BF16)
            with tc.high_priority():
                for c in range(4):
                    nc.vector.tensor_copy(xb_t[:, c, :], xT_t[:, c, :])
                nc.vector.tensor_copy(wb_t, wT_t)

            # ---------------- qT/kT = (x @ W.T).T halves  [64, S] ----------
            # band products are staged into [64, 1026] rows with col c
            # holding index i = c-1 (cols 0, 1024, 1025 are don't-care pads
            # feeding only masked/unused lanes).
            qT_t = setup.tile([64, S], F32)
            kT_t = setup.tile([64, S], F32)
            tu_t = setup.tile([64, 1026], F32)
            tl_t = setup.tile([64, 1026], F32)
            ps_diff = psd.tile([128, 3, NB], F32)
            with tc.high_priority():
                # pads: col c holds score index c-1.  s_u[1023] and s_l[0]
                # must be -inf-ish so the edge rows' one-neighbor softmax
                # saturates (a_u[0]=1, a_l[1023]=1); the MM sums 64 copies,
                # still hugely negative.  Cols 0/1025 feed only unused lanes.
                NEG = -1e30
                nc.vector.memset(tu_t[:, 0:1], 0.0)
                nc.vector.memset(tu_t[:, 1024:1025], NEG)
                nc.vector.memset(tu_t[:, 1025:1026], 0.0)
                nc.vector.memset(tl_t[:, 0:1], 0.0)
                nc.vector.memset(tl_t[:, 1:2], NEG)
                nc.vector.memset(tl_t[:, 1025:1026], 0.0)
                for j in range(2):
                    for half, dest_t in enumerate((qT_t, kT_t)):
                        ps = mm.tile([64, 512], F32, tag="mmbig")
                        for c in range(4):
                            nc.tensor.matmul(
                                ps[:],
                                lhsT=wb_t[:, c, half * 64:(half + 1) * 64],
                                rhs=xb_t[:, c, j * 512:(j + 1) * 512],
                                start=(c == 0),
                                stop=(c == 3),
                            )
                        nc.vector.tensor_scalar_add(
                            dest_t[:, j * 512:(j + 1) * 512], ps,
                            bias_t[half * 64:(half + 1) * 64, 0:1])
                    # band products, staged so col c holds score index c-1:
                    #   tu[1+i] = s_u[i] = q_i.k_{i+1}     (i in [0,1023))
                    #   tl[2+i] = s_l[i+1] = q_{i+1}.k_i   -> tl[c] = s_l[c-1]
                    lo, hi = (0, 511) if j == 0 else (511, 1023)
                    nc.vector.tensor_mul(tu_t[:, 1 + lo:1 + hi],
                                         qT_t[:, lo:hi],
                                         kT_t[:, lo + 1:hi + 1])
                    nc.vector.tensor_mul(tl_t[:, 2 + lo:2 + hi],
                                         qT_t[:, lo + 1:hi + 1],
                                         kT_t[:, lo:hi])

                # banded score diffs straight into [128, 3, NB] tile layout:
                # ps_diff[p, g, r] = s_u[i+g-1] - s_l[i+g-1],  i = 128r + p,
                # via paired matmuls: (tu2 slice).T @ ones + (tl2 slice).T @
                # -ones accumulated into one PSUM column.
                for g in range(3):
                    for r in range(NB):
                        c0 = 128 * r + g
                        nc.tensor.matmul(ps_diff[:, g, r:r + 1],
                                         lhsT=tu_t[:, c0:c0 + 128],
                                         rhs=ones_col[0:64, :],
                                         start=True, stop=False)
                        nc.tensor.matmul(ps_diff[:, g, r:r + 1],
                                         lhsT=tl_t[:, c0:c0 + 128],
                                         rhs=nones_col[0:64, :],
                                         start=False, stop=True)

                # 2-element softmax via sigmoid on all 3 offset groups at once
                a_u = setup.tile([128, 3, NB], F32)
                nc.scalar.activation(a_u, ps_diff, AF.Sigmoid, scale=1.0 / E)
                a_l = setup.tile([128, 3, NB], F32)
                nc.scalar.activation(a_l, ps_diff, AF.Sigmoid, scale=-1.0 / E)

                # g_l[i] = g_u[i-1] = sqrt(a_u[i-1]*a_l[i] + eps)  (cols 0:8)
                # g_u[i]            = sqrt(a_u[i]*a_l[i+1] + eps)  (cols 8:16)
                gq_t = setup.tile([128, 2, NB], F32)
                nc.vector.tensor_mul(gq_t, _ap(a_u[:], 0, [[24, 128], [8, 2], [1, NB]]),
                                     _ap(a_l[:], 8, [[24, 128], [8, 2], [1, NB]]))
                g_t = setup.tile([128, 2, NB], F32)
                nc.scalar.activation(g_t, gq_t, AF.Sqrt, bias=eps_t[:, 0:1])

                # na band values as per-row vectors:
                #   na_b2[:,0,:] = na[i,i-1] = g_l + pr_l*(1-g_l)
                #   na_b2[:,1,:] = na[i,i+1] = g_u + pr_u*(1-g_u)
                omg2 = setup.tile([128, 2, NB], F32)
                nc.vector.tensor_scalar(omg2, g_t, -1.0, 1.0, op0=ALU.mult,
                                        op1=ALU.add)
                prm = setup.tile([128, 2, NB], F32)
                nc.vector.tensor_mul(prm, pr_lu, omg2)
                na_b2 = setup.tile([128, 2, NB], F32)
                nc.vector.tensor_add(na_b2, prm, g_t)

                # u = ln(na[i,i+1] + eps);  nd = na[i,i];  lnnd = ln(nd+eps)
                u_t = setup.tile([128, NB], F32)
                nc.scalar.activation(u_t, na_b2[:, 1, :], AF.Ln,
                                     bias=eps_t[:, 0:1])
                nd_t = setup.tile([128, NB], F32)
                nc.vector.tensor_scalar(nd_t, pr_d, 1.0 - C0, C0,
                                        op0=ALU.mult, op1=ALU.add)
                lnnd = setup.tile([128, NB], F32)
                nc.scalar.activation(lnnd, nd_t, AF.Ln, bias=eps_t[:, 0:1])
                # preload the Exp table while DVE runs the U dance
                nc.scalar.activation(warm_t, eps_t[0:1, 0:1], AF.Exp,
                                     scale=-1.0)

                # band-correction scalars vs the bulk na formula:
                #   na_band - na_bulk = (g - C0) * (1 - prior)  at [i, i-/+1]
                pu1_2 = setup.tile([128, 2, NB], F32)
                nc.vector.tensor_scalar(pu1_2, pr_lu, -1.0, 1.0, op0=ALU.mult,
                                        op1=ALU.add)
                gc_2 = setup.tile([128, 2, NB], F32)
                nc.vector.tensor_scalar(gc_2, g_t, C0, None, op0=ALU.subtract)
                cu_2 = setup.tile([128, 2, NB], F32)
                nc.vector.tensor_mul(cu_2, gc_2, pu1_2)

                # ---- U = exclusive prefix sum of u ----
                inc_ps = ps_small.tile([128, NB], F32, tag="tiny")
                nc.tensor.matmul(inc_ps, lhsT=triu_t, rhs=u_t, start=True,
                                 stop=True)
                exc_t = setup.tile([128, NB], F32)
                nc.vector.tensor_sub(exc_t, inc_ps, u_t)

                cs_ps = ps_small.tile([1, NB], F32, tag="tiny")  # block sums
                nc.tensor.matmul(cs_ps, lhsT=ones_col, rhs=u_t, start=True,
                                 stop=True)
                bp_t = setup.tile([1, NB], F32)
                nc.vector.memset(bp_t[:, 0:1], 0.0)
                nc.vector.tensor_copy(bp_t[:, 1:8], cs_ps[0:1, 0:7])
                zer_t = setup.tile([1, NB], F32)
                nc.vector.memset(zer_t, 0.0)
                bpx_t = setup.tile([1, NB], F32)      # exclusive block prefix
                nc.vector.tensor_tensor_scan(bpx_t, bp_t, zer_t, 0.0,
                                             op0=ALU.add, op1=ALU.add)
                bpr_ps = ps_small.tile([128, NB], F32, tag="tiny")
                nc.tensor.matmul(bpr_ps, lhsT=ones_row, rhs=bpx_t, start=True,
                                 stop=True)
                U_t = setup.tile([128, NB], F32)
                nc.vector.tensor_add(U_t, exc_t, bpr_ps)

                # U -> row layout via identity-matmul transpose (no DMA):
                # out[0, n] = sum_k U[k, r] * I[k, n] = U[n, r]
                ur_ps = psu.tile([1, S], F32, tag="urow")
                for r in range(NB):
                    nc.tensor.matmul(ur_ps[0:1, r * 128:(r + 1) * 128],
                                     lhsT=U_t[:, r:r + 1], rhs=ident_t,
                                     start=True, stop=True)
                U_lin = setup.tile([1, S], F32)
                nc.vector.tensor_copy(U_lin, ur_ps)
                # U_rep[p, j] = U[j] via ones broadcast matmul
                Ur_ps = psrep.tile([128, S], F32, tag="urep")
                for lo in (0, 512):
                    nc.tensor.matmul(Ur_ps[:, lo:lo + 512], lhsT=ones_row,
                                     rhs=U_lin[0:1, lo:lo + 512], start=True,
                                     stop=True)

            # ---------------- pass 1: prior loads + na full rows ----------
            pr_ts = []
            for r in range(NB):
                pr_t = prp.tile([128, S], F32, tag="pr")
                nc.sync.dma_start(out=pr_t, in_=prior[r * 128:(r + 1) * 128, :])
                pr_ts.append(pr_t)
            for r in range(NB):
                na_t = napool.tile([128, S], F32, tag="na")
                nc.vector.tensor_scalar(na_t, pr_ts[r], 1.0 - C0, C0,
                                        op0=ALU.mult, op1=ALU.add)
                nc.sync.dma_start(out=na_out[r * 128:(r + 1) * 128, :],
                                  in_=na_t)

            # ---------------- pass 2: c_attn rows ----------------
            # cd = |U[j] - U[i]|.  U is non-increasing (u < 0), so left of
            # the diagonal window d >= 0 and right of it d <= 0: one
            # tensor_scalar per region gives |d| directly; a true abs (max
            # of +/-d) is only needed in the 130-wide diagonal window.
            # The diag is pre-patched so exp(-cd) lands na[i,i] there.
            for r in range(NB):
                w0 = r * 128 - 1
                wlo = max(w0, 0)
                whi = min(w0 + 130, S)
                wd = whi - wlo
                mo = wlo - w0
                Ui = U_t[:, r:r + 1]

                cd_t = cdpool.tile([128, S], F32, tag="cd")
                if wlo > 0:
                    nc.vector.tensor_scalar(cd_t[:, 0:wlo], Ur_ps[:, 0:wlo],
                                            Ui, None, op0=ALU.subtract)
                if whi < S:
                    nc.vector.tensor_scalar(cd_t[:, whi:S], Ur_ps[:, whi:S],
                                            Ui, -1.0, op0=ALU.subtract,
                                            op1=ALU.mult)
                ta_t = cdpool.tile([128, 130], F32, tag="ta")
                nc.vector.tensor_scalar(ta_t[:, :wd], Ur_ps[:, wlo:whi],
                                        Ui, None, op0=ALU.subtract)
                tb_t = cdpool.tile([128, 130], F32, tag="tb")
                nc.vector.tensor_scalar(tb_t[:, :wd], Ur_ps[:, wlo:whi],
                                        Ui, -1.0, op0=ALU.subtract,
                                        op1=ALU.mult)
                t5w = cdpool.tile([128, 130], F32, tag="t5w")
                nc.vector.tensor_scalar(t5w[:, :wd], md_t[:, mo:mo + wd],
                                        lnnd[:, r:r + 1], None, op0=ALU.mult)
                nc.vector.tensor_max(cd_t[:, wlo:whi], ta_t[:, :wd],
                                     tb_t[:, :wd])
                nc.vector.tensor_sub(cd_t[:, wlo:whi], cd_t[:, wlo:whi],
                                     t5w[:, :wd])
                c2_t = c2pool.tile([128, S], F32, tag="c2")
                nc.scalar.activation(c2_t, cd_t, AF.Exp, scale=-1.0)
                nc.scalar.dma_start(out=c_out[r * 128:(r + 1) * 128, :],
                                    in_=c2_t)

            # ---------------- band overwrite (tail; tiny stores) ----------
            # rebuild the [128,130] window from pr_t with the off-diagonal
            # corrections added, stored as 130-wide rows (520 B/descriptor).
            # Same SP ring as the bulk na stores -> FIFO gives WAW order.
            for r in range(NB):
                w0 = r * 128 - 1
                wlo = max(w0, 0)
                whi = min(w0 + 130, S)
                wd = whi - wlo
                mo = wlo - w0
                bw_t = napool.tile([128, 130], F32, tag="bw")
                nc.vector.tensor_scalar(bw_t[:, :wd], pr_ts[r][:, wlo:whi],
                                        1.0 - C0, C0, op0=ALU.mult,
                                        op1=ALU.add)
                t1w = napool.tile([128, 130], F32, tag="t1w")
                nc.vector.tensor_scalar(t1w[:, :wd], mu_t[:, mo:mo + wd],
                                        cu_2[:, 1, r:r + 1], None,
                                        op0=ALU.mult)
                nc.vector.tensor_add(bw_t[:, :wd], bw_t[:, :wd], t1w[:, :wd])
                t2w = napool.tile([128, 130], F32, tag="t2w")
                nc.vector.tensor_scalar(t2w[:, :wd], ml_t[:, mo:mo + wd],
                                        cu_2[:, 0, r:r + 1], None,
                                        op0=ALU.mult)
                nc.vector.tensor_add(bw_t[:, :wd], bw_t[:, :wd], t2w[:, :wd])
                nc.sync.dma_start(
                    out=_ap(na_out, r * 128 * S + wlo, [[S, 128], [1, wd]]),
                    in_=bw_t[:, :wd])

    _split_multi_waits(nc)
    return nc


def _get_nc():
    if "nc" not in _CACHE:
        _CACHE["nc"] = build_nc()
    return _CACHE["nc"]


def run(inputs, trace=False, tmpdir=None):
    nc = _get_nc()
    context = np.asarray(inputs["context"], np.float32)
    prior = np.asarray(inputs["prior"], np.float32)
    w = np.asarray(inputs["proj_weight"], np.float32)
    bias = np.asarray(inputs["proj_bias"], np.float32)

    wT = np.ascontiguousarray(w.T)                     # [E, 2P]
    bcol = np.ascontiguousarray(bias.reshape(P2, 1))
    in_maps = []
    for b in range(B):
        in_maps.append({
            "xT": np.ascontiguousarray(context[:, b, :].T),   # [E, S]
            "wT": wT,
            "bvec": bcol,
            "prior": np.ascontiguousarray(prior[b]),
        })
    try:
        res = run_bass_kernel_spmd(nc, in_maps, list(range(B)), trace=trace,
                                   tmpdir=tmpdir)
    except ModuleNotFoundError:
        res = run_bass_kernel_spmd(nc, in_maps, list(range(B)), trace=False)
    c = np.stack([res.results[i]["c_out"] for i in range(B)])
    na = np.stack([res.results[i]["na_out"] for i in range(B)])
    return (c, na), res


def kernel(**inputs):
    (c, na), _ = run(inputs)
    return (c, na)


# revision 45
# speedup vs baseline: 2.4735x; 1.0429x over previous
"""ConstituentAttention Trainium2 kernel.

Math (derived from the reference):
  - score is masked to the super/sub-diagonal only, so the row softmax is a
    2-element softmax: a_u[i] = sigmoid((s_u[i]-s_l[i])/E), a_l = 1-a_u,
    where s_u[i] = q_i.k_{i+1}, s_l[i] = q_i.k_{i-1}.
  - neighbor_attn = prior + (1-prior)*g where g == sqrt(1e-9) =: C0 everywhere
    except g[i,i+1] = g[i+1,i] = sqrt(a_u[i]*a_l[i+1] + 1e-9) =: g_u[i].
  - log-space prefix products collapse to c_attn[i,j] = exp(-|U[j]-U[i]|) for
    i != j, where U = exclusive prefix sum of u_i = log(na[i,i+1] + 1e-9);
    diagonal of c_attn = na[i,i].

Sharding: data-parallel over batch, one batch element per NeuronCore (B=8).

Engine discipline (v1 lesson): DVE 2-port ops and GpSimd compute take an
exclusive lock on the shared SBUF port pair - concurrent DVE+GpSimd work
stretches BOTH ~15x.  All elementwise compute lives on Vector, activations
on Scalar, GpSimd only issues the early scatter-gather DMAs.

Data-movement discipline (v2/v3 lessons):
  - diag-scatter stores (4-12 B descriptors) grind the SDMA engines with HBM
    read-modify-writes; the band overwrite uses 130-wide row strips instead.
  - DRAM staging round trips for cross-partition reshapes stall 10-20 us
    behind bulk traffic.  ALL reshapes now ride the PE array: the banded
    score differences s_u[i+d]-s_l[i+d] land directly in [128, NB] layout
    via 48 tiny matmuls (lhsT = shifted 128-col slices of the product rows,
    rhs = +/-ones accumulating in PSUM), and U[128,NB] -> row layout goes
    through identity-matmul transposes.  Zero staging DMAs.
  - bf16 cast on DVE for the qk matmuls (scores are /E then sigmoided; bf16
    noise lands ~1e-4 on the outputs, the gate is 2e-2).
"""

import numpy as np

import concourse.bass as bass
import concourse.tile as tile
from concourse import mybir
from concourse.bass_utils import run_bass_kernel_spmd

S, B, E, P = 1024, 8, 512, 64
P2 = 2 * P
NB = S // 128
C0 = float(np.sqrt(1e-9))
F32 = mybir.dt.float32
BF16 = mybir.dt.bfloat16
AF = mybir.ActivationFunctionType
ALU = mybir.AluOpType

_CACHE = {}


def _ap(handle_or_ap, offset, dims):
    a0 = handle_or_ap[:] if not isinstance(handle_or_ap, bass.AP) else handle_or_ap
    return bass.AP(tensor=a0.tensor, offset=offset, ap=[list(d) for d in dims])


def _split_multi_waits(nc):
    """This toolchain's walrus accepts at most ONE embedded on_wait per
    instruction; hoist extras into standalone EventSemaphore waits just
    before the instruction on the same engine."""
    n = 0
    for bb in nc.main_func.blocks:
        new = []
        for ins in bb.instructions:
            si = ins.sync_info
            if si is not None and si.on_wait and len(si.on_wait) > 1:
                for w in si.on_wait[:-1]:
                    n += 1
                    wi = mybir.InstEventSemaphore(
                        name=f"I-waitsplit-{n}",
                        opcode="EventSemaphore",
                        engine=ins.engine,
                        sync_info=mybir.SyncInfo(on_wait=[w], on_update=[]),
                    )
                    try:
                        nc.register_instruction(wi)
                    except Exception:
                        pass
                    new.append(wi)
                si.on_wait = si.on_wait[-1:]
            new.append(ins)
        try:
            bb.instructions[:] = new
        except TypeError:
            bb.instructions = new
    return n


def build_nc():
    nc = bass.Bass()

    xT = nc.dram_tensor("xT", [E, S], F32, kind="ExternalInput")
    wT = nc.dram_tensor("wT", [E, P2], F32, kind="ExternalInput")
    bvec = nc.dram_tensor("bvec", [P2, 1], F32, kind="ExternalInput")
    prior = nc.dram_tensor("prior", [S, S], F32, kind="ExternalInput")
    na_out = nc.dram_tensor("na_out", [S, S], F32, kind="ExternalOutput")
    c_out = nc.dram_tensor("c_out", [S, S], F32, kind="ExternalOutput")

    # masks [128,130]: for row-block r the band lives in absolute cols
    # [r*128-1, r*128+129); with window origin w0 = r*128-1 the diag sits at
    # rel col p+1, super at p+2, sub at p, independent of r.
    p_i = np.arange(128)[:, None]
    c_i = np.arange(130)[None, :]
    md_h = nc.inline_tensor((c_i == p_i + 1).astype(np.float32), "mask_d")
    mu_h = nc.inline_tensor((c_i == p_i + 2).astype(np.float32), "mask_u")
    ml_h = nc.inline_tensor((c_i == p_i).astype(np.float32), "mask_l")
    # lhsT for within-block inclusive cumsum over partitions: out = triu.T @ u
    triu_h = nc.inline_tensor(
        np.triu(np.ones((128, 128), np.float32)), "triu_ones"
    )
    ident_h = nc.inline_tensor(np.eye(128, dtype=np.float32), "ident")
    ones_col_h = nc.inline_tensor(np.ones((128, 1), np.float32), "ones_col")
    ones_row_h = nc.inline_tensor(np.ones((1, 128), np.float32), "ones_row")
    import ml_dtypes
    ones_cb_h = nc.inline_tensor(np.ones((64, 1), ml_dtypes.bfloat16),
                                 "ones_cb")
    nones_cb_h = nc.inline_tensor(np.full((64, 1), -1.0, ml_dtypes.bfloat16),
                                  "nones_cb")

    with tile.TileContext(nc) as tc:
        with (
            tc.tile_pool(name="setup", bufs=1) as setup,
            tc.tile_pool(name="na", bufs=4) as napool,
            tc.tile_pool(name="cdp", bufs=3) as cdpool,
            tc.tile_pool(name="c2p", bufs=3) as c2pool,
            tc.tile_pool(name="prp", bufs=8) as prp,
            tc.tile_pool(name="mm", bufs=2, space="PSUM") as mm,
            tc.tile_pool(name="ps_small", bufs=1, space="PSUM") as ps_small,
            tc.tile_pool(name="psu", bufs=1, space="PSUM") as psu,
            tc.tile_pool(name="psd", bufs=1, space="PSUM") as psd,
            tc.tile_pool(name="psrep", bufs=1, space="PSUM") as psrep,
        ):
            # -------- critical-path loads first on SP: xT chunks + wT -------
            xT_t = setup.tile([128, 4, S], F32)
            wT_t = setup.tile([128, 4, P2], F32)
            bias_t = setup.tile([128, 1], F32)
            nc.sync.dma_start(
                out=xT_t[:, 0, :], in_=_ap(xT, 0, [[S, 128], [1, S]]))
            nc.sync.dma_start(
                out=wT_t,
                in_=_ap(wT, 0, [[P2, 128], [128 * P2, 4], [1, P2]]))
            nc.sync.dma_start(out=bias_t, in_=bvec[:])
            for c in range(1, 4):
                nc.sync.dma_start(
                    out=xT_t[:, c, :],
                    in_=_ap(xT, c * 128 * S, [[S, 128], [1, S]]))

            # ------- prior band gathers -------------------------------------
            # On the SP ring BEHIND the xT chunks and AHEAD of the prior
            # bulk loads: their thousands of 4 B descriptors would otherwise
            # round-robin against xT and starve the whole U chain (SWDGE got
            # only a 1/3 bandwidth share in v3/v4 traces).
            # pr_lu[:, 0, :] = prior[i, i-1] (row 0 unused -> 0)
            # pr_lu[:, 1, :] = prior[i, i+1] (row 1023 unused -> 0)
            pr_lu = setup.tile([128, 2, NB], F32)
            nc.vector.memset(pr_lu[0:1, 0, 0:1], 0.0)
            nc.vector.memset(pr_lu[:, 1, 7:8], 0.0)
            nc.sync.dma_start(
                out=pr_lu[1:128, 0, 0:1],
                in_=_ap(prior, S, [[S + 1, 127], [1, 1]]))
            nc.sync.dma_start(
                out=pr_lu[:, 0, 1:8],
                in_=_ap(prior, 128 * (S + 1) - 1,
                        [[S + 1, 128], [128 * (S + 1), 7]]))
            nc.sync.dma_start(
                out=pr_lu[:, 1, 0:7],
                in_=_ap(prior, 1, [[S + 1, 128], [128 * (S + 1), 7]]))
            nc.sync.dma_start(
                out=pr_lu[0:127, 1, 7:8],
                in_=_ap(prior, 896 * (S + 1) + 1, [[S + 1, 127], [1, 1]]))
            pr_d = setup.tile([128, NB], F32)            # prior[i, i]
            nc.sync.dma_start(
                out=pr_d, in_=_ap(prior, 0, [[S + 1, 128], [128 * (S + 1), 8]]))

            # small hot constants via the Act queue; the big masks/triu are
            # dehoisted below the qk section so their ~330 KB of small
            # descriptors don't round-robin against the xT chunks at t=0
            ones_col = setup.tile([128, 1], F32)
            nc.scalar.dma_start(out=ones_col, in_=ones_col_h[:])
            ones_row = setup.tile([1, 128], F32)
            nc.scalar.dma_start(out=ones_row, in_=ones_row_h[:])
            ones_cb = setup.tile([64, 1], BF16)
            nc.scalar.dma_start(out=ones_cb, in_=ones_cb_h[:])
            nones_cb = setup.tile([64, 1], BF16)
            nc.scalar.dma_start(out=nones_cb, in_=nones_cb_h[:])

            # preload the Sigmoid activation table during the idle head
            eps_t = setup.tile([128, 1], F32)
            c0_t = setup.tile([128, 1], F32)
            with tc.high_priority():
                nc.vector.memset(eps_t, 1e-9)
                nc.vector.memset(c0_t, C0)
            warm_t = setup.tile([1, 1], F32)
            with tc.high_priority():
                nc.scalar.activation(warm_t, eps_t[0:1, 0:1], AF.Sigmoid)

            # bf16 copies of x/W for the qk matmuls (DVE is idle this early)
            xb_t = setup.tile([128, 4, S], BF16)
            wb_t = setup.tile([128, 4, P2], BF16)
            with tc.high_priority():
                for c in range(4):
                    nc.vector.tensor_copy(xb_t[:, c, :], xT_t[:, c, :])
                nc.vector.tensor_copy(wb_t, wT_t)

            # ---------------- qT/kT = (x @ W.T).T halves  [64, S] ----------
            # band products are staged into [64, 1026] rows with col c
            # holding index i = c-1 (cols 0, 1024, 1025 are don't-care pads
            # feeding only masked/unused lanes).
            qT_t = setup.tile([64, S], F32)
            kT_t = setup.tile([64, S], F32)
            # bf16 product rows: the 48 banded-diff matmuls use 128-col
            # slices of these as lhsT - bf16 gets FWL (4x faster LDWEIGHTS)
            tu_t = setup.tile([64, 1026], BF16)
            tl_t = setup.tile([64, 1026], BF16)
            ps_diff = psd.tile([128, 3, NB], F32)
            with tc.high_priority():
                # pads: col c holds score index c-1.  s_u[1023] and s_l[0]
                # must be -inf-ish so the edge rows' one-neighbor softmax
                # saturates (a_u[0]=1, a_l[1023]=1); the MM sums 64 copies,
                # still hugely negative.  Cols 0/1025 feed only unused lanes.
                NEG = -1e30
                nc.vector.memset(tu_t[:, 0:1], 0.0)
                nc.vector.memset(tu_t[:, 1024:1025], NEG)
                nc.vector.memset(tu_t[:, 1025:1026], 0.0)
                nc.vector.memset(tl_t[:, 0:1], 0.0)
                nc.vector.memset(tl_t[:, 1:2], NEG)
                nc.vector.memset(tl_t[:, 1025:1026], 0.0)
                for j in range(2):
                    for half, dest_t in enumerate((qT_t, kT_t)):
                        ps = mm.tile([64, 512], F32, tag="mmbig")
                        for c in range(4):
                            nc.tensor.matmul(
                                ps[:],
                                lhsT=wb_t[:, c, half * 64:(half + 1) * 64],
                                rhs=xb_t[:, c, j * 512:(j + 1) * 512],
                                start=(c == 0),
                                stop=(c == 3),
                            )
                        nc.vector.tensor_scalar_add(
                            dest_t[:, j * 512:(j + 1) * 512], ps,
                            bias_t[half * 64:(half + 1) * 64, 0:1])
                    # band products, staged so col c holds score index c-1:
                    #   tu[1+i] = s_u[i] = q_i.k_{i+1}     (i in [0,1023))
                    #   tl[2+i] = s_l[i+1] = q_{i+1}.k_i   -> tl[c] = s_l[c-1]
                    lo, hi = (0, 511) if j == 0 else (511, 1023)
                    nc.vector.tensor_mul(tu_t[:, 1 + lo:1 + hi],
                                         qT_t[:, lo:hi],
                                         kT_t[:, lo + 1:hi + 1])
                    nc.vector.tensor_mul(tl_t[:, 2 + lo:2 + hi],
                                         qT_t[:, lo + 1:hi + 1],
                                         kT_t[:, lo:hi])

                # dehoisted bulky constants (needed from the U-dance onward)
                md_t = setup.tile([128, 130], F32)
                nc.scalar.dma_start(out=md_t, in_=md_h[:])
                mu_t = setup.tile([128, 130], F32)
                nc.scalar.dma_start(out=mu_t, in_=mu_h[:])
                ml_t = setup.tile([128, 130], F32)
                nc.scalar.dma_start(out=ml_t, in_=ml_h[:])
                triu_t = setup.tile([128, 128], F32)
                nc.scalar.dma_start(out=triu_t, in_=triu_h[:])
                ident_t = setup.tile([128, 128], F32)
                nc.scalar.dma_start(out=ident_t, in_=ident_h[:])

                # banded score diffs straight into [128, 3, NB] tile layout:
                # ps_diff[p, g, r] = s_u[i+g-1] - s_l[i+g-1],  i = 128r + p,
                # via paired matmuls: (tu2 slice).T @ ones + (tl2 slice).T @
                # -ones accumulated into one PSUM column.
                for g in range(3):
                    for r in range(NB):
                        c0 = 128 * r + g
                        nc.tensor.matmul(ps_diff[:, g, r:r + 1],
                                         lhsT=tu_t[:, c0:c0 + 128],
                                         rhs=ones_cb,
                                         start=True, stop=False)
                        nc.tensor.matmul(ps_diff[:, g, r:r + 1],
                                         lhsT=tl_t[:, c0:c0 + 128],
                                         rhs=nones_cb,
                                         start=False, stop=True)

                # 2-element softmax via sigmoid on all 3 offset groups at once
                a_u = setup.tile([128, 3, NB], F32)
                nc.scalar.activation(a_u, ps_diff, AF.Sigmoid, scale=1.0 / E)
                a_l = setup.tile([128, 3, NB], F32)
                nc.scalar.activation(a_l, ps_diff, AF.Sigmoid, scale=-1.0 / E)

                # g_l[i] = g_u[i-1] = sqrt(a_u[i-1]*a_l[i] + eps)  (cols 0:8)
                # g_u[i]            = sqrt(a_u[i]*a_l[i+1] + eps)  (cols 8:16)
                gq_t = setup.tile([128, 2, NB], F32)
                nc.vector.tensor_mul(gq_t, _ap(a_u[:], 0, [[24, 128], [8, 2], [1, NB]]),
                                     _ap(a_l[:], 8, [[24, 128], [8, 2], [1, NB]]))
                g_t = setup.tile([128, 2, NB], F32)
                nc.scalar.activation(g_t, gq_t, AF.Sqrt, bias=eps_t[:, 0:1])

                # na band values as per-row vectors:
                #   na_b2[:,0,:] = na[i,i-1] = g_l + pr_l*(1-g_l)
                #   na_b2[:,1,:] = na[i,i+1] = g_u + pr_u*(1-g_u)
                omg2 = setup.tile([128, 2, NB], F32)
                nc.vector.tensor_scalar(omg2, g_t, -1.0, 1.0, op0=ALU.mult,
                                        op1=ALU.add)
                prm = setup.tile([128, 2, NB], F32)
                nc.vector.tensor_mul(prm, pr_lu, omg2)
                # lnin: col 1 = na[i,i+1], col 0 = nd = na[i,i], so ONE Ln
                # produces both u and ln(nd) (avoids an extra ACT table swap)
                lnin = setup.tile([128, 2, NB], F32)
                nc.vector.tensor_add(lnin[:, 1, :], prm[:, 1, :], g_t[:, 1, :])
                nc.vector.tensor_scalar(lnin[:, 0, :], pr_d, 1.0 - C0, C0,
                                        op0=ALU.mult, op1=ALU.add)
                nd_t = lnin[:, 0, :]
                u2 = setup.tile([128, 2, NB], F32)
                nc.scalar.activation(u2, lnin, AF.Ln, bias=eps_t[:, 0:1])
                u_t = u2[:, 1, :]
                lnnd = u2[:, 0, :]
                # preload the Exp table; gated on u2 so it cannot be hoisted
                # between the Sigmoid/Sqrt/Ln uses and thrash the table RAM
                nc.scalar.activation(warm_t, u2[0:1, 0, 0:1], AF.Exp,
                                     scale=-1.0)

                # band-correction scalars vs the bulk na formula:
                #   na_band - na_bulk = (g - C0) * (1 - prior)  at [i, i-/+1]
                pu1_2 = setup.tile([128, 2, NB], F32)
                nc.vector.tensor_scalar(pu1_2, pr_lu, -1.0, 1.0, op0=ALU.mult,
                                        op1=ALU.add)
                gc_2 = setup.tile([128, 2, NB], F32)
                nc.vector.tensor_scalar(gc_2, g_t, C0, None, op0=ALU.subtract)
                cu_2 = setup.tile([128, 2, NB], F32)
                nc.vector.tensor_mul(cu_2, gc_2, pu1_2)

                # ---- U = exclusive prefix sum of u ----
                inc_ps = ps_small.tile([128, NB], F32, tag="tiny")
                nc.tensor.matmul(inc_ps, lhsT=triu_t, rhs=u_t, start=True,
                                 stop=True)
                exc_t = setup.tile([128, NB], F32)
                nc.vector.tensor_sub(exc_t, inc_ps, u_t)

                cs_ps = ps_small.tile([1, NB], F32, tag="tiny")  # block sums
                nc.tensor.matmul(cs_ps, lhsT=ones_col, rhs=u_t, start=True,
                                 stop=True)
                bp_t = setup.tile([1, NB], F32)
                nc.vector.memset(bp_t[:, 0:1], 0.0)
                nc.vector.tensor_copy(bp_t[:, 1:8], cs_ps[0:1, 0:7])
                zer_t = setup.tile([1, NB], F32)
                nc.vector.memset(zer_t, 0.0)
                bpx_t = setup.tile([1, NB], F32)      # exclusive block prefix
                nc.vector.tensor_tensor_scan(bpx_t, bp_t, zer_t, 0.0,
                                             op0=ALU.add, op1=ALU.add)
                bpr_ps = ps_small.tile([128, NB], F32, tag="tiny")
                nc.tensor.matmul(bpr_ps, lhsT=ones_row, rhs=bpx_t, start=True,
                                 stop=True)
                U_t = setup.tile([128, NB], F32)
                nc.vector.tensor_add(U_t, exc_t, bpr_ps)
                negU_t = setup.tile([128, NB], F32)
                nc.vector.tensor_scalar(negU_t, U_t, -1.0, None, op0=ALU.mult)

                # U -> row layout via identity-matmul transpose (no DMA):
                # out[0, n] = sum_k U[k, r] * I[k, n] = U[n, r]
                ur_ps = psu.tile([1, S], F32, tag="urow")
                for r in range(NB):
                    nc.tensor.matmul(ur_ps[0:1, r * 128:(r + 1) * 128],
                                     lhsT=U_t[:, r:r + 1], rhs=ident_t,
                                     start=True, stop=True)
                U_lin = setup.tile([1, S], F32)
                nc.scalar.activation(U_lin, ur_ps, AF.Identity)
                Ur_ps = psrep.tile([128, S], F32, tag="urep")
                for lo in (0, 512):
                    nc.tensor.matmul(Ur_ps[:, lo:lo + 512], lhsT=ones_row,
                                     rhs=U_lin[0:1, lo:lo + 512], start=True,
                                     stop=True)

            # ---------------- pass 1: prior loads + na full rows ----------
            pr_ts = []
            for r in range(NB):
                pr_t = prp.tile([128, S], F32, tag="pr")
                nc.sync.dma_start(out=pr_t, in_=prior[r * 128:(r + 1) * 128, :])
                pr_ts.append(pr_t)
            for r in range(NB):
                na_t = napool.tile([128, S], F32, tag="na")
                nc.vector.tensor_scalar(na_t, pr_ts[r], 1.0 - C0, C0,
                                        op0=ALU.mult, op1=ALU.add)
                nc.sync.dma_start(out=na_out[r * 128:(r + 1) * 128, :],
                                  in_=na_t)

            # ---------------- pass 2: c_attn rows ----------------
            # cd = |U[j] - U[i]|.  U is non-increasing (u < 0), so left of
            # the diagonal window d >= 0 and right of it d <= 0: one
            # tensor_scalar per region gives |d| directly; a true abs (max
            # of +/-d) is only needed in the 130-wide diagonal window.
            # The diag is pre-patched so exp(-cd) lands na[i,i] there.
            for r in range(NB):
                w0 = r * 128 - 1
                wlo = max(w0, 0)
                whi = min(w0 + 130, S)
                wd = whi - wlo
                mo = wlo - w0
                Ui = U_t[:, r:r + 1]

                # |d| left region on ACT (affine w/ per-partition bias),
                # right region on DVE - balances the two engines; DVE also
                # handles the 130-wide ambiguous mid window
                cd_t = cdpool.tile([128, S], F32, tag="cd")
                if wlo > 0:
                    nc.scalar.activation(cd_t[:, 0:wlo], Ur_ps[:, 0:wlo],
                                         AF.Identity,
                                         bias=negU_t[:, r:r + 1])
                if whi < S:
                    nc.vector.tensor_scalar(cd_t[:, whi:S], Ur_ps[:, whi:S],
                                            Ui, -1.0, op0=ALU.subtract,
                                            op1=ALU.mult)
                ta_t = cdpool.tile([128, 130], F32, tag="ta")
                nc.vector.tensor_scalar(ta_t[:, :wd], Ur_ps[:, wlo:whi],
                                        Ui, None, op0=ALU.subtract)
                tb_t = cdpool.tile([128, 130], F32, tag="tb")
                nc.vector.tensor_scalar(tb_t[:, :wd], Ur_ps[:, wlo:whi],
                                        Ui, -1.0, op0=ALU.subtract,
                                        op1=ALU.mult)
                t5w = cdpool.tile([128, 130], F32, tag="t5w")
                nc.vector.tensor_scalar(t5w[:, :wd], md_t[:, mo:mo + wd],
                                        lnnd[:, r:r + 1], None, op0=ALU.mult)
                nc.vector.tensor_max(cd_t[:, wlo:whi], ta_t[:, :wd],
                                     tb_t[:, :wd])
                nc.vector.tensor_sub(cd_t[:, wlo:whi], cd_t[:, wlo:whi],
                                     t5w[:, :wd])
                c2_t = c2pool.tile([128, S], F32, tag="c2")
                nc.scalar.activation(c2_t, cd_t, AF.Exp, scale=-1.0)
                nc.scalar.dma_start(out=c_out[r * 128:(r + 1) * 128, :],
                                    in_=c2_t)

            # ---------------- band overwrite (tail; tiny stores) ----------
            # rebuild the [128,130] window from pr_t with the off-diagonal
            # corrections added, stored as 130-wide rows (520 B/descriptor).
            # Same SP ring as the bulk na stores -> FIFO gives WAW order.
            for r in range(NB):
                w0 = r * 128 - 1
                wlo = max(w0, 0)
                whi = min(w0 + 130, S)
                wd = whi - wlo
                mo = wlo - w0
                bw_t = napool.tile([128, 130], F32, tag="bw")
                nc.scalar.activation(bw_t[:, :wd], pr_ts[r][:, wlo:whi],
                                     AF.Identity, scale=1.0 - C0,
                                     bias=c0_t[:, 0:1])
                t1w = napool.tile([128, 130], F32, tag="t1w")
                nc.vector.tensor_scalar(t1w[:, :wd], mu_t[:, mo:mo + wd],
                                        cu_2[:, 1, r:r + 1], None,
                                        op0=ALU.mult)
                nc.vector.tensor_add(bw_t[:, :wd], bw_t[:, :wd], t1w[:, :wd])
                t2w = napool.tile([128, 130], F32, tag="t2w")
                nc.vector.tensor_scalar(t2w[:, :wd], ml_t[:, mo:mo + wd],
                                        cu_2[:, 0, r:r + 1], None,
                                        op0=ALU.mult)
                nc.vector.tensor_add(bw_t[:, :wd], bw_t[:, :wd], t2w[:, :wd])
                nc.sync.dma_start(
                    out=_ap(na_out, r * 128 * S + wlo, [[S, 128], [1, wd]]),
                    in_=bw_t[:, :wd])

    _split_multi_waits(nc)
    return nc


def _get_nc():
    if "nc" not in _CACHE:
        _CACHE["nc"] = build_nc()
    return _CACHE["nc"]


def run(inputs, trace=False, tmpdir=None):
    nc = _get_nc()
    context = np.asarray(inputs["context"], np.float32)
    prior = np.asarray(inputs["prior"], np.float32)
    w = np.asarray(inputs["proj_weight"], np.float32)
    bias = np.asarray(inputs["proj_bias"], np.float32)

    wT = np.ascontiguousarray(w.T)                     # [E, 2P]
    bcol = np.ascontiguousarray(bias.reshape(P2, 1))
    in_maps = []
    for b in range(B):
        in_maps.append({
            "xT": np.ascontiguousarray(context[:, b, :].T),   # [E, S]
            "wT": wT,
            "bvec": bcol,
            "prior": np.ascontiguousarray(prior[b]),
        })
    try:
        res = run_bass_kernel_spmd(nc, in_maps, list(range(B)), trace=trace,
                                   tmpdir=tmpdir)
    except ModuleNotFoundError:
        res = run_bass_kernel_spmd(nc, in_maps, list(range(B)), trace=False)
    c = np.stack([res.results[i]["c_out"] for i in range(B)])
    na = np.stack([res.results[i]["na_out"] for i in range(B)])
    return (c, na), res


def kernel(**inputs):
    (c, na), _ = run(inputs)
    return (c, na)


# revision 49
# speedup vs baseline: 2.7350x; 1.1057x over previous
"""ConstituentAttention Trainium2 kernel.

Math (derived from the reference):
  - score is masked to the super/sub-diagonal only, so the row softmax is a
    2-element softmax: a_u[i] = sigmoid((s_u[i]-s_l[i])/E), a_l = 1-a_u,
    where s_u[i] = q_i.k_{i+1}, s_l[i] = q_i.k_{i-1}.
  - neighbor_attn = prior + (1-prior)*g where g == sqrt(1e-9) =: C0 everywhere
    except g[i,i+1] = g[i+1,i] = sqrt(a_u[i]*a_l[i+1] + 1e-9) =: g_u[i].
  - log-space prefix products collapse to c_attn[i,j] = exp(-|U[j]-U[i]|) for
    i != j, where U = exclusive prefix sum of u_i = log(na[i,i+1] + 1e-9);
    diagonal of c_attn = na[i,i].

Sharding: data-parallel over batch, one batch element per NeuronCore (B=8).

Engine discipline (v1 lesson): DVE 2-port ops and GpSimd compute take an
exclusive lock on the shared SBUF port pair - concurrent DVE+GpSimd work
stretches BOTH ~15x.  All elementwise compute lives on Vector, activations
on Scalar, GpSimd only issues the early scatter-gather DMAs.

Data-movement discipline (v2/v3 lessons):
  - diag-scatter stores (4-12 B descriptors) grind the SDMA engines with HBM
    read-modify-writes; the band overwrite uses 130-wide row strips instead.
  - DRAM staging round trips for cross-partition reshapes stall 10-20 us
    behind bulk traffic.  ALL reshapes now ride the PE array: the banded
    score differences s_u[i+d]-s_l[i+d] land directly in [128, NB] layout
    via 48 tiny matmuls (lhsT = shifted 128-col slices of the product rows,
    rhs = +/-ones accumulating in PSUM), and U[128,NB] -> row layout goes
    through identity-matmul transposes.  Zero staging DMAs.
  - bf16 cast on DVE for the qk matmuls (scores are /E then sigmoided; bf16
    noise lands ~1e-4 on the outputs, the gate is 2e-2).
"""

import numpy as np

import concourse.bass as bass
import concourse.tile as tile
from concourse import mybir
from concourse.bass_utils import run_bass_kernel_spmd

S, B, E, P = 1024, 8, 512, 64
P2 = 2 * P
NB = S // 128
C0 = float(np.sqrt(1e-9))
F32 = mybir.dt.float32
BF16 = mybir.dt.bfloat16
AF = mybir.ActivationFunctionType
ALU = mybir.AluOpType

_CACHE = {}


def _ap(handle_or_ap, offset, dims):
    a0 = handle_or_ap[:] if not isinstance(handle_or_ap, bass.AP) else handle_or_ap
    return bass.AP(tensor=a0.tensor, offset=offset, ap=[list(d) for d in dims])


def _split_multi_waits(nc):
    """This toolchain's walrus accepts at most ONE embedded on_wait per
    instruction; hoist extras into standalone EventSemaphore waits just
    before the instruction on the same engine."""
    n = 0
    for bb in nc.main_func.blocks:
        new = []
        for ins in bb.instructions:
            si = ins.sync_info
            if si is not None and si.on_wait and len(si.on_wait) > 1:
                for w in si.on_wait[:-1]:
                    n += 1
                    wi = mybir.InstEventSemaphore(
                        name=f"I-waitsplit-{n}",
                        opcode="EventSemaphore",
                        engine=ins.engine,
                        sync_info=mybir.SyncInfo(on_wait=[w], on_update=[]),
                    )
                    try:
                        nc.register_instruction(wi)
                    except Exception:
                        pass
                    new.append(wi)
                si.on_wait = si.on_wait[-1:]
            new.append(ins)
        try:
            bb.instructions[:] = new
        except TypeError:
            bb.instructions = new
    return n


def build_nc():
    nc = bass.Bass()

    xT = nc.dram_tensor("xT", [E, S], F32, kind="ExternalInput")
    wT = nc.dram_tensor("wT", [E, P2], F32, kind="ExternalInput")
    bvec = nc.dram_tensor("bvec", [P2, 1], F32, kind="ExternalInput")
    prior = nc.dram_tensor("prior", [S, S], F32, kind="ExternalInput")
    na_out = nc.dram_tensor("na_out", [S, S], F32, kind="ExternalOutput")
    c_out = nc.dram_tensor("c_out", [S, S], F32, kind="ExternalOutput")

    # masks [128,130]: for row-block r the band lives in absolute cols
    # [r*128-1, r*128+129); with window origin w0 = r*128-1 the diag sits at
    # rel col p+1, super at p+2, sub at p, independent of r.
    p_i = np.arange(128)[:, None]
    c_i = np.arange(130)[None, :]
    md_h = nc.inline_tensor((c_i == p_i + 1).astype(np.float32), "mask_d")
    mu_h = nc.inline_tensor((c_i == p_i + 2).astype(np.float32), "mask_u")
    ml_h = nc.inline_tensor((c_i == p_i).astype(np.float32), "mask_l")
    # lhsT for within-block inclusive cumsum over partitions: out = triu.T @ u
    triu_h = nc.inline_tensor(
        np.triu(np.ones((128, 128), np.float32)), "triu_ones"
    )
    ident_h = nc.inline_tensor(np.eye(128, dtype=np.float32), "ident")
    ones_col_h = nc.inline_tensor(np.ones((128, 1), np.float32), "ones_col")
    ones_row_h = nc.inline_tensor(np.ones((1, 128), np.float32), "ones_row")
    import ml_dtypes
    ones_cb_h = nc.inline_tensor(np.ones((64, 1), ml_dtypes.bfloat16),
                                 "ones_cb")
    nones_cb_h = nc.inline_tensor(np.full((64, 1), -1.0, ml_dtypes.bfloat16),
                                  "nones_cb")

    with tile.TileContext(nc) as tc:
        with (
            tc.tile_pool(name="setup", bufs=1) as setup,
            tc.tile_pool(name="na", bufs=4) as napool,
            tc.tile_pool(name="cdp", bufs=3) as cdpool,
            tc.tile_pool(name="c2p", bufs=3) as c2pool,
            tc.tile_pool(name="prp", bufs=8) as prp,
            tc.tile_pool(name="mm", bufs=2, space="PSUM") as mm,
            tc.tile_pool(name="ps_small", bufs=1, space="PSUM") as ps_small,
            tc.tile_pool(name="psu", bufs=1, space="PSUM") as psu,
            tc.tile_pool(name="psd", bufs=1, space="PSUM") as psd,
            tc.tile_pool(name="psrep", bufs=1, space="PSUM") as psrep,
        ):
            # -------- critical-path loads first on SP: xT chunks + wT -------
            xT_t = setup.tile([128, 4, S], F32)
            wT_t = setup.tile([128, 4, P2], F32)
            bias_t = setup.tile([128, 1], F32)
            nc.sync.dma_start(
                out=xT_t[:, 0, :], in_=_ap(xT, 0, [[S, 128], [1, S]]))
            nc.sync.dma_start(
                out=wT_t,
                in_=_ap(wT, 0, [[P2, 128], [128 * P2, 4], [1, P2]]))
            nc.sync.dma_start(out=bias_t, in_=bvec[:])
            for c in range(1, 4):
                nc.sync.dma_start(
                    out=xT_t[:, c, :],
                    in_=_ap(xT, c * 128 * S, [[S, 128], [1, S]]))

            # ------- prior bulk loads + band gathers on the SP ring ---------
            # FIFO order: xT chunks first (they gate the whole U chain),
            # two prior blocks, then the gathers (thousands of 4 B
            # descriptors - anywhere earlier they round-robin against xT and
            # starve it), then the remaining prior blocks.
            pr_ts = []
            for r in range(2):
                pr_t = prp.tile([128, S], F32, tag="pr")
                nc.sync.dma_start(out=pr_t, in_=prior[r * 128:(r + 1) * 128, :])
                pr_ts.append(pr_t)
            # pr_lu[:, 0, :] = prior[i, i-1] (row 0 unused -> 0)
            # pr_lu[:, 1, :] = prior[i, i+1] (row 1023 unused -> 0)
            pr_lu = setup.tile([128, 2, NB], F32)
            nc.vector.memset(pr_lu[0:1, 0, 0:1], 0.0)
            nc.vector.memset(pr_lu[:, 1, 7:8], 0.0)
            nc.sync.dma_start(
                out=pr_lu[1:128, 0, 0:1],
                in_=_ap(prior, S, [[S + 1, 127], [1, 1]]))
            nc.sync.dma_start(
                out=pr_lu[:, 0, 1:8],
                in_=_ap(prior, 128 * (S + 1) - 1,
                        [[S + 1, 128], [128 * (S + 1), 7]]))
            nc.sync.dma_start(
                out=pr_lu[:, 1, 0:7],
                in_=_ap(prior, 1, [[S + 1, 128], [128 * (S + 1), 7]]))
            nc.sync.dma_start(
                out=pr_lu[0:127, 1, 7:8],
                in_=_ap(prior, 896 * (S + 1) + 1, [[S + 1, 127], [1, 1]]))
            pr_d = setup.tile([128, NB], F32)            # prior[i, i]
            nc.sync.dma_start(
                out=pr_d, in_=_ap(prior, 0, [[S + 1, 128], [128 * (S + 1), 8]]))
            for r in range(2, NB):
                pr_t = prp.tile([128, S], F32, tag="pr")
                nc.sync.dma_start(out=pr_t, in_=prior[r * 128:(r + 1) * 128, :])
                pr_ts.append(pr_t)

            # small hot constants via the Act queue; the big masks/triu are
            # dehoisted below the qk section so their ~330 KB of small
            # descriptors don't round-robin against the xT chunks at t=0
            ones_col = setup.tile([128, 1], F32)
            nc.scalar.dma_start(out=ones_col, in_=ones_col_h[:])
            ones_row = setup.tile([1, 128], F32)
            nc.scalar.dma_start(out=ones_row, in_=ones_row_h[:])
            ones_cb = setup.tile([64, 1], BF16)
            nc.scalar.dma_start(out=ones_cb, in_=ones_cb_h[:])
            nones_cb = setup.tile([64, 1], BF16)
            nc.scalar.dma_start(out=nones_cb, in_=nones_cb_h[:])

            # preload the Sigmoid activation table during the idle head
            eps_t = setup.tile([128, 1], F32)
            c0_t = setup.tile([128, 1], F32)
            with tc.high_priority():
                nc.vector.memset(eps_t, 1e-9)
                nc.vector.memset(c0_t, C0)
            warm_t = setup.tile([1, 1], F32)
            with tc.high_priority():
                nc.scalar.activation(warm_t, eps_t[0:1, 0:1], AF.Sigmoid)

            # bf16 copies of x/W for the qk matmuls (DVE is idle this early)
            xb_t = setup.tile([128, 4, S], BF16)
            wb_t = setup.tile([128, 4, P2], BF16)
            with tc.high_priority():
                for c in range(4):
                    nc.vector.tensor_copy(xb_t[:, c, :], xT_t[:, c, :])
                nc.vector.tensor_copy(wb_t, wT_t)

            # ---------------- qT/kT = (x @ W.T).T halves  [64, S] ----------
            # band products are staged into [64, 1026] rows with col c
            # holding index i = c-1 (cols 0, 1024, 1025 are don't-care pads
            # feeding only masked/unused lanes).
            qT_t = setup.tile([64, S], F32)
            kT_t = setup.tile([64, S], F32)
            # bf16 product rows: the 48 banded-diff matmuls use 128-col
            # slices of these as lhsT - bf16 gets FWL (4x faster LDWEIGHTS)
            tu_t = setup.tile([64, 1026], BF16)
            tl_t = setup.tile([64, 1026], BF16)
            ps_diff = psd.tile([128, 3, NB], F32)
            with tc.high_priority():
                # pads: col c holds score index c-1.  s_u[1023] and s_l[0]
                # must be -inf-ish so the edge rows' one-neighbor softmax
                # saturates (a_u[0]=1, a_l[1023]=1); the MM sums 64 copies,
                # still hugely negative.  Cols 0/1025 feed only unused lanes.
                NEG = -1e30
                nc.vector.memset(tu_t[:, 0:1], 0.0)
                nc.vector.memset(tu_t[:, 1024:1025], NEG)
                nc.vector.memset(tu_t[:, 1025:1026], 0.0)
                nc.vector.memset(tl_t[:, 0:1], 0.0)
                nc.vector.memset(tl_t[:, 1:2], NEG)
                nc.vector.memset(tl_t[:, 1025:1026], 0.0)
                for j in range(2):
                    for half, dest_t in enumerate((qT_t, kT_t)):
                        ps = mm.tile([64, 512], F32, tag="mmbig")
                        for c in range(4):
                            nc.tensor.matmul(
                                ps[:],
                                lhsT=wb_t[:, c, half * 64:(half + 1) * 64],
                                rhs=xb_t[:, c, j * 512:(j + 1) * 512],
                                start=(c == 0),
                                stop=(c == 3),
                            )
                        nc.vector.tensor_scalar_add(
                            dest_t[:, j * 512:(j + 1) * 512], ps,
                            bias_t[half * 64:(half + 1) * 64, 0:1])
                    # band products, staged so col c holds score index c-1:
                    #   tu[1+i] = s_u[i] = q_i.k_{i+1}     (i in [0,1023))
                    #   tl[2+i] = s_l[i+1] = q_{i+1}.k_i   -> tl[c] = s_l[c-1]
                    lo, hi = (0, 511) if j == 0 else (511, 1023)
                    nc.vector.tensor_mul(tu_t[:, 1 + lo:1 + hi],
                                         qT_t[:, lo:hi],
                                         kT_t[:, lo + 1:hi + 1])
                    nc.vector.tensor_mul(tl_t[:, 2 + lo:2 + hi],
                                         qT_t[:, lo + 1:hi + 1],
                                         kT_t[:, lo:hi])

                # dehoisted bulky constants (needed from the U-dance onward)
                md_t = setup.tile([128, 130], F32)
                nc.scalar.dma_start(out=md_t, in_=md_h[:])
                mu_t = setup.tile([128, 130], F32)
                nc.scalar.dma_start(out=mu_t, in_=mu_h[:])
                ml_t = setup.tile([128, 130], F32)
                nc.scalar.dma_start(out=ml_t, in_=ml_h[:])
                triu_t = setup.tile([128, 128], F32)
                nc.scalar.dma_start(out=triu_t, in_=triu_h[:])
                ident_t = setup.tile([128, 128], F32)
                nc.scalar.dma_start(out=ident_t, in_=ident_h[:])

                # banded score diffs straight into [128, 3, NB] tile layout:
                # ps_diff[p, g, r] = s_u[i+g-1] - s_l[i+g-1],  i = 128r + p,
                # via paired matmuls: (tu2 slice).T @ ones + (tl2 slice).T @
                # -ones accumulated into one PSUM column.
                for g in range(3):
                    for r in range(NB):
                        c0 = 128 * r + g
                        nc.tensor.matmul(ps_diff[:, g, r:r + 1],
                                         lhsT=tu_t[:, c0:c0 + 128],
                                         rhs=ones_cb,
                                         start=True, stop=False)
                        nc.tensor.matmul(ps_diff[:, g, r:r + 1],
                                         lhsT=tl_t[:, c0:c0 + 128],
                                         rhs=nones_cb,
                                         start=False, stop=True)

                # 2-element softmax via sigmoid on all 3 offset groups at once
                a_u = setup.tile([128, 3, NB], F32)
                nc.scalar.activation(a_u, ps_diff, AF.Sigmoid, scale=1.0 / E)
                a_l = setup.tile([128, 3, NB], F32)
                nc.scalar.activation(a_l, ps_diff, AF.Sigmoid, scale=-1.0 / E)

                # g_l[i] = g_u[i-1] = sqrt(a_u[i-1]*a_l[i] + eps)  (cols 0:8)
                # g_u[i]            = sqrt(a_u[i]*a_l[i+1] + eps)  (cols 8:16)
                gq_t = setup.tile([128, 2, NB], F32)
                nc.vector.tensor_mul(gq_t, _ap(a_u[:], 0, [[24, 128], [8, 2], [1, NB]]),
                                     _ap(a_l[:], 8, [[24, 128], [8, 2], [1, NB]]))
                g_t = setup.tile([128, 2, NB], F32)
                nc.scalar.activation(g_t, gq_t, AF.Sqrt, bias=eps_t[:, 0:1])

                # na band values as per-row vectors:
                #   na_b2[:,0,:] = na[i,i-1] = g_l + pr_l*(1-g_l)
                #   na_b2[:,1,:] = na[i,i+1] = g_u + pr_u*(1-g_u)
                omg2 = setup.tile([128, 2, NB], F32)
                nc.vector.tensor_scalar(omg2, g_t, -1.0, 1.0, op0=ALU.mult,
                                        op1=ALU.add)
                prm = setup.tile([128, 2, NB], F32)
                nc.vector.tensor_mul(prm, pr_lu, omg2)
                # lnin: col 1 = na[i,i+1], col 0 = nd = na[i,i], so ONE Ln
                # produces both u and ln(nd) (avoids an extra ACT table swap)
                lnin = setup.tile([128, 2, NB], F32)
                nc.vector.tensor_add(lnin[:, 1, :], prm[:, 1, :], g_t[:, 1, :])
                nc.vector.tensor_scalar(lnin[:, 0, :], pr_d, 1.0 - C0, C0,
                                        op0=ALU.mult, op1=ALU.add)
                nd_t = lnin[:, 0, :]
                u2 = setup.tile([128, 2, NB], F32)
                nc.scalar.activation(u2, lnin, AF.Ln, bias=eps_t[:, 0:1])
                u_t = u2[:, 1, :]
                lnnd = u2[:, 0, :]
                # preload the Exp table; gated on u2 so it cannot be hoisted
                # between the Sigmoid/Sqrt/Ln uses and thrash the table RAM
                nc.scalar.activation(warm_t, u2[0:1, 0, 0:1], AF.Exp,
                                     scale=-1.0)

                # band-correction scalars vs the bulk na formula:
                #   na_band - na_bulk = (g - C0) * (1 - prior)  at [i, i-/+1]
                pu1_2 = setup.tile([128, 2, NB], F32)
                nc.vector.tensor_scalar(pu1_2, pr_lu, -1.0, 1.0, op0=ALU.mult,
                                        op1=ALU.add)
                gc_2 = setup.tile([128, 2, NB], F32)
                nc.vector.tensor_scalar(gc_2, g_t, C0, None, op0=ALU.subtract)
                cu_2 = setup.tile([128, 2, NB], F32)
                nc.vector.tensor_mul(cu_2, gc_2, pu1_2)

                # ---- U = exclusive prefix sum of u ----
                inc_ps = ps_small.tile([128, NB], F32, tag="tiny")
                nc.tensor.matmul(inc_ps, lhsT=triu_t, rhs=u_t, start=True,
                                 stop=True)
                exc_t = setup.tile([128, NB], F32)
                nc.vector.tensor_sub(exc_t, inc_ps, u_t)

                cs_ps = ps_small.tile([1, NB], F32, tag="tiny")  # block sums
                nc.tensor.matmul(cs_ps, lhsT=ones_col, rhs=u_t, start=True,
                                 stop=True)
                bp_t = setup.tile([1, NB], F32)
                nc.vector.memset(bp_t[:, 0:1], 0.0)
                nc.vector.tensor_copy(bp_t[:, 1:8], cs_ps[0:1, 0:7])
                zer_t = setup.tile([1, NB], F32)
                nc.vector.memset(zer_t, 0.0)
                bpx_t = setup.tile([1, NB], F32)      # exclusive block prefix
                nc.vector.tensor_tensor_scan(bpx_t, bp_t, zer_t, 0.0,
                                             op0=ALU.add, op1=ALU.add)
                bpr_ps = ps_small.tile([128, NB], F32, tag="tiny")
                nc.tensor.matmul(bpr_ps, lhsT=ones_row, rhs=bpx_t, start=True,
                                 stop=True)
                U_t = setup.tile([128, NB], F32)
                nc.vector.tensor_add(U_t, exc_t, bpr_ps)
                negU_t = setup.tile([128, NB], F32)
                nc.vector.tensor_scalar(negU_t, U_t, -1.0, None, op0=ALU.mult)

                # U -> row layout via identity-matmul transpose (no DMA):
                # out[0, n] = sum_k U[k, r] * I[k, n] = U[n, r]
                ur_ps = psu.tile([1, S], F32, tag="urow")
                for r in range(NB):
                    nc.tensor.matmul(ur_ps[0:1, r * 128:(r + 1) * 128],
                                     lhsT=U_t[:, r:r + 1], rhs=ident_t,
                                     start=True, stop=True)
                U_lin = setup.tile([1, S], F32)
                nc.scalar.activation(U_lin, ur_ps, AF.Identity)
                Ur_ps = psrep.tile([128, S], F32, tag="urep")
                for lo in (0, 512):
                    nc.tensor.matmul(Ur_ps[:, lo:lo + 512], lhsT=ones_row,
                                     rhs=U_lin[0:1, lo:lo + 512], start=True,
                                     stop=True)

            # ---------------- pass 1: na full rows ----------
            for r in range(NB):
                na_t = napool.tile([128, S], F32, tag="na")
                nc.vector.tensor_scalar(na_t, pr_ts[r], 1.0 - C0, C0,
                                        op0=ALU.mult, op1=ALU.add)
                nc.sync.dma_start(out=na_out[r * 128:(r + 1) * 128, :],
                                  in_=na_t)

            # ---------------- pass 2: c_attn rows ----------------
            # cd = |U[j] - U[i]|.  U is non-increasing (u < 0), so left of
            # the diagonal window d >= 0 and right of it d <= 0: one
            # tensor_scalar per region gives |d| directly; a true abs (max
            # of +/-d) is only needed in the 130-wide diagonal window.
            # The diag is pre-patched so exp(-cd) lands na[i,i] there.
            for r in range(NB):
                w0 = r * 128 - 1
                wlo = max(w0, 0)
                whi = min(w0 + 130, S)
                wd = whi - wlo
                mo = wlo - w0
                Ui = U_t[:, r:r + 1]

                # |d| left/right regions on ACT (affine w/ per-partition
                # bias); DVE handles the 130-wide ambiguous mid window plus
                # the na band rebuild - measured as the balanced split
                cd_t = cdpool.tile([128, S], F32, tag="cd")
                if wlo > 0:
                    nc.scalar.activation(cd_t[:, 0:wlo], Ur_ps[:, 0:wlo],
                                         AF.Identity,
                                         bias=negU_t[:, r:r + 1])
                if whi < S:
                    nc.scalar.activation(cd_t[:, whi:S], Ur_ps[:, whi:S],
                                         AF.Identity, scale=-1.0,
                                         bias=Ui)
                ta_t = cdpool.tile([128, 130], F32, tag="ta")
                nc.vector.tensor_scalar(ta_t[:, :wd], Ur_ps[:, wlo:whi],
                                        Ui, None, op0=ALU.subtract)
                tb_t = cdpool.tile([128, 130], F32, tag="tb")
                nc.vector.tensor_scalar(tb_t[:, :wd], Ur_ps[:, wlo:whi],
                                        Ui, -1.0, op0=ALU.subtract,
                                        op1=ALU.mult)
                t5w = cdpool.tile([128, 130], F32, tag="t5w")
                nc.vector.tensor_scalar(t5w[:, :wd], md_t[:, mo:mo + wd],
                                        lnnd[:, r:r + 1], None, op0=ALU.mult)
                nc.vector.tensor_max(cd_t[:, wlo:whi], ta_t[:, :wd],
                                     tb_t[:, :wd])
                nc.vector.tensor_sub(cd_t[:, wlo:whi], cd_t[:, wlo:whi],
                                     t5w[:, :wd])
                c2_t = c2pool.tile([128, S], F32, tag="c2")
                nc.scalar.activation(c2_t, cd_t, AF.Exp, scale=-1.0)
                nc.scalar.dma_start(out=c_out[r * 128:(r + 1) * 128, :],
                                    in_=c2_t)

            # ---------------- band overwrite (tail; tiny stores) ----------
            # rebuild the [128,130] window from pr_t with the off-diagonal
            # corrections added, stored as 130-wide rows (520 B/descriptor).
            # Same SP ring as the bulk na stores -> FIFO gives WAW order.
            for r in range(NB):
                w0 = r * 128 - 1
                wlo = max(w0, 0)
                whi = min(w0 + 130, S)
                wd = whi - wlo
                mo = wlo - w0
                bw_t = napool.tile([128, 130], F32, tag="bw")
                nc.vector.tensor_scalar(bw_t[:, :wd], pr_ts[r][:, wlo:whi],
                                        1.0 - C0, C0, op0=ALU.mult,
                                        op1=ALU.add)
                t1w = napool.tile([128, 130], F32, tag="t1w")
                nc.vector.tensor_scalar(t1w[:, :wd], mu_t[:, mo:mo + wd],
                                        cu_2[:, 1, r:r + 1], None,
                                        op0=ALU.mult)
                nc.vector.tensor_add(bw_t[:, :wd], bw_t[:, :wd], t1w[:, :wd])
                t2w = napool.tile([128, 130], F32, tag="t2w")
                nc.vector.tensor_scalar(t2w[:, :wd], ml_t[:, mo:mo + wd],
                                        cu_2[:, 0, r:r + 1], None,
                                        op0=ALU.mult)
                nc.vector.tensor_add(bw_t[:, :wd], bw_t[:, :wd], t2w[:, :wd])
                nc.sync.dma_start(
                    out=_ap(na_out, r * 128 * S + wlo, [[S, 128], [1, wd]]),
                    in_=bw_t[:, :wd])

    _split_multi_waits(nc)
    return nc


def _get_nc():
    if "nc" not in _CACHE:
        _CACHE["nc"] = build_nc()
    return _CACHE["nc"]


def run(inputs, trace=False, tmpdir=None):
    nc = _get_nc()
    context = np.asarray(inputs["context"], np.float32)
    prior = np.asarray(inputs["prior"], np.float32)
    w = np.asarray(inputs["proj_weight"], np.float32)
    bias = np.asarray(inputs["proj_bias"], np.float32)

    wT = np.ascontiguousarray(w.T)                     # [E, 2P]
    bcol = np.ascontiguousarray(bias.reshape(P2, 1))
    in_maps = []
    for b in range(B):
        in_maps.append({
            "xT": np.ascontiguousarray(context[:, b, :].T),   # [E, S]
            "wT": wT,
            "bvec": bcol,
            "prior": np.ascontiguousarray(prior[b]),
        })
    try:
        res = run_bass_kernel_spmd(nc, in_maps, list(range(B)), trace=trace,
                                   tmpdir=tmpdir)
    except ModuleNotFoundError:
        res = run_bass_kernel_spmd(nc, in_maps, list(range(B)), trace=False)
    c = np.stack([res.results[i]["c_out"] for i in range(B)])
    na = np.stack([res.results[i]["na_out"] for i in range(B)])
    return (c, na), res


def kernel(**inputs):
    (c, na), _ = run(inputs)
    return (c, na)


# revision 53
# speedup vs baseline: 3.0363x; 1.1101x over previous
"""ConstituentAttention Trainium2 kernel.

Math (derived from the reference):
  - score is masked to the super/sub-diagonal only, so the row softmax is a
    2-element softmax: a_u[i] = sigmoid((s_u[i]-s_l[i])/E), a_l = 1-a_u,
    where s_u[i] = q_i.k_{i+1}, s_l[i] = q_i.k_{i-1}.
  - neighbor_attn = prior + (1-prior)*g where g == sqrt(1e-9) =: C0 everywhere
    except g[i,i+1] = g[i+1,i] = sqrt(a_u[i]*a_l[i+1] + 1e-9) =: g_u[i].
  - log-space prefix products collapse to c_attn[i,j] = exp(-|U[j]-U[i]|) for
    i != j, where U = exclusive prefix sum of u_i = log(na[i,i+1] + 1e-9);
    diagonal of c_attn = na[i,i].

Sharding: data-parallel over batch, one batch element per NeuronCore (B=8).

Engine discipline (v1 lesson): DVE 2-port ops and GpSimd compute take an
exclusive lock on the shared SBUF port pair - concurrent DVE+GpSimd work
stretches BOTH ~15x.  All elementwise compute lives on Vector, activations
on Scalar, GpSimd only issues the early scatter-gather DMAs.

Data-movement discipline (v2/v3 lessons):
  - diag-scatter stores (4-12 B descriptors) grind the SDMA engines with HBM
    read-modify-writes; the band overwrite uses 130-wide row strips instead.
  - DRAM staging round trips for cross-partition reshapes stall 10-20 us
    behind bulk traffic.  ALL reshapes now ride the PE array: the banded
    score differences s_u[i+d]-s_l[i+d] land directly in [128, NB] layout
    via 48 tiny matmuls (lhsT = shifted 128-col slices of the product rows,
    rhs = +/-ones accumulating in PSUM), and U[128,NB] -> row layout goes
    through identity-matmul transposes.  Zero staging DMAs.
  - bf16 cast on DVE for the qk matmuls (scores are /E then sigmoided; bf16
    noise lands ~1e-4 on the outputs, the gate is 2e-2).
"""

import numpy as np

import concourse.bass as bass
import concourse.tile as tile
from concourse import mybir
from concourse.bass_utils import run_bass_kernel_spmd

S, B, E, P = 1024, 8, 512, 64
P2 = 2 * P
NB = S // 128
C0 = float(np.sqrt(1e-9))
F32 = mybir.dt.float32
BF16 = mybir.dt.bfloat16
AF = mybir.ActivationFunctionType
ALU = mybir.AluOpType

_CACHE = {}


def _ap(handle_or_ap, offset, dims):
    a0 = handle_or_ap[:] if not isinstance(handle_or_ap, bass.AP) else handle_or_ap
    return bass.AP(tensor=a0.tensor, offset=offset, ap=[list(d) for d in dims])


def _split_multi_waits(nc):
    """This toolchain's walrus accepts at most ONE embedded on_wait per
    instruction; hoist extras into standalone EventSemaphore waits just
    before the instruction on the same engine."""
    n = 0
    for bb in nc.main_func.blocks:
        new = []
        for ins in bb.instructions:
            si = ins.sync_info
            if si is not None and si.on_wait and len(si.on_wait) > 1:
                for w in si.on_wait[:-1]:
                    n += 1
                    wi = mybir.InstEventSemaphore(
                        name=f"I-waitsplit-{n}",
                        opcode="EventSemaphore",
                        engine=ins.engine,
                        sync_info=mybir.SyncInfo(on_wait=[w], on_update=[]),
                    )
                    try:
                        nc.register_instruction(wi)
                    except Exception:
                        pass
                    new.append(wi)
                si.on_wait = si.on_wait[-1:]
            new.append(ins)
        try:
            bb.instructions[:] = new
        except TypeError:
            bb.instructions = new
    return n


def build_nc():
    nc = bass.Bass()

    xT = nc.dram_tensor("xT", [E, S], F32, kind="ExternalInput")
    wT = nc.dram_tensor("wT", [E, P2], F32, kind="ExternalInput")
    bvec = nc.dram_tensor("bvec", [P2, 1], F32, kind="ExternalInput")
    prior = nc.dram_tensor("prior", [S, S], F32, kind="ExternalInput")
    na_out = nc.dram_tensor("na_out", [S, S], F32, kind="ExternalOutput")
    c_out = nc.dram_tensor("c_out", [S, S], F32, kind="ExternalOutput")

    # masks [128,130]: for row-block r the band lives in absolute cols
    # [r*128-1, r*128+129); with window origin w0 = r*128-1 the diag sits at
    # rel col p+1, super at p+2, sub at p, independent of r.
    p_i = np.arange(128)[:, None]
    c_i = np.arange(130)[None, :]
    md_h = nc.inline_tensor((c_i == p_i + 1).astype(np.float32), "mask_d")
    mu_h = nc.inline_tensor((c_i == p_i + 2).astype(np.float32), "mask_u")
    ml_h = nc.inline_tensor((c_i == p_i).astype(np.float32), "mask_l")
    # lhsT for within-block inclusive cumsum over partitions: out = triu.T @ u
    triu_h = nc.inline_tensor(
        np.triu(np.ones((128, 128), np.float32)), "triu_ones"
    )
    ident_h = nc.inline_tensor(np.eye(128, dtype=np.float32), "ident")
    ones_col_h = nc.inline_tensor(np.ones((128, 1), np.float32), "ones_col")
    ones_row_h = nc.inline_tensor(np.ones((1, 128), np.float32), "ones_row")
    import ml_dtypes
    ones_cb_h = nc.inline_tensor(np.ones((64, 1), ml_dtypes.bfloat16),
                                 "ones_cb")
    nones_cb_h = nc.inline_tensor(np.full((64, 1), -1.0, ml_dtypes.bfloat16),
                                  "nones_cb")

    with tile.TileContext(nc) as tc:
        with (
            tc.tile_pool(name="setup", bufs=1) as setup,
            tc.tile_pool(name="na", bufs=4) as napool,
            tc.tile_pool(name="cdp", bufs=3) as cdpool,
            tc.tile_pool(name="c2p", bufs=3) as c2pool,
            tc.tile_pool(name="prp", bufs=8) as prp,
            tc.tile_pool(name="mm", bufs=2, space="PSUM") as mm,
            tc.tile_pool(name="ps_small", bufs=1, space="PSUM") as ps_small,
            tc.tile_pool(name="psu", bufs=1, space="PSUM") as psu,
            tc.tile_pool(name="psd", bufs=1, space="PSUM") as psd,
            tc.tile_pool(name="psrep", bufs=1, space="PSUM") as psrep,
        ):
            # -------- critical-path loads first on SP: xT chunks + wT -------
            xT_t = setup.tile([128, 4, S], F32)
            wT_t = setup.tile([128, 4, P2], F32)
            bias_t = setup.tile([128, 1], F32)
            nc.sync.dma_start(
                out=xT_t[:, 0, :], in_=_ap(xT, 0, [[S, 128], [1, S]]))
            nc.sync.dma_start(
                out=wT_t,
                in_=_ap(wT, 0, [[P2, 128], [128 * P2, 4], [1, P2]]))
            nc.sync.dma_start(out=bias_t, in_=bvec[:])
            for c in range(1, 4):
                nc.sync.dma_start(
                    out=xT_t[:, c, :],
                    in_=_ap(xT, c * 128 * S, [[S, 128], [1, S]]))

            # ------- prior bulk loads + band gathers on the SP ring ---------
            # FIFO order: xT chunks first (they gate the whole U chain),
            # two prior blocks, then the gathers (thousands of 4 B
            # descriptors - anywhere earlier they round-robin against xT and
            # starve it), then the remaining prior blocks.
            pr_ts = []
            for r in range(2):
                pr_t = prp.tile([128, S], F32, tag="pr")
                nc.sync.dma_start(out=pr_t, in_=prior[r * 128:(r + 1) * 128, :])
                pr_ts.append(pr_t)
            # pr_lu[:, 0, :] = prior[i, i-1] (row 0 unused -> 0)
            # pr_lu[:, 1, :] = prior[i, i+1] (row 1023 unused -> 0)
            pr_lu = setup.tile([128, 2, NB], F32)
            nc.vector.memset(pr_lu[0:1, 0, 0:1], 0.0)
            nc.vector.memset(pr_lu[:, 1, 7:8], 0.0)
            nc.sync.dma_start(
                out=pr_lu[1:128, 0, 0:1],
                in_=_ap(prior, S, [[S + 1, 127], [1, 1]]))
            nc.sync.dma_start(
                out=pr_lu[:, 0, 1:8],
                in_=_ap(prior, 128 * (S + 1) - 1,
                        [[S + 1, 128], [128 * (S + 1), 7]]))
            nc.sync.dma_start(
                out=pr_lu[:, 1, 0:7],
                in_=_ap(prior, 1, [[S + 1, 128], [128 * (S + 1), 7]]))
            nc.sync.dma_start(
                out=pr_lu[0:127, 1, 7:8],
                in_=_ap(prior, 896 * (S + 1) + 1, [[S + 1, 127], [1, 1]]))
            pr_d = setup.tile([128, NB], F32)            # prior[i, i]
            nc.sync.dma_start(
                out=pr_d, in_=_ap(prior, 0, [[S + 1, 128], [128 * (S + 1), 8]]))
            for r in range(2, NB):
                pr_t = prp.tile([128, S], F32, tag="pr")
                nc.sync.dma_start(out=pr_t, in_=prior[r * 128:(r + 1) * 128, :])
                pr_ts.append(pr_t)
            # bulky constants on the SP ring: FIFO places them behind the
            # latency-critical loads (the Act ring can't guarantee this -
            # Tile hoists its issues by readiness, and they then round-robin
            # against the xT chunks)
            md_t = setup.tile([128, 130], F32)
            nc.sync.dma_start(out=md_t, in_=md_h[:])
            mu_t = setup.tile([128, 130], F32)
            nc.sync.dma_start(out=mu_t, in_=mu_h[:])
            ml_t = setup.tile([128, 130], F32)
            nc.sync.dma_start(out=ml_t, in_=ml_h[:])
            triu_t = setup.tile([128, 128], F32)
            nc.sync.dma_start(out=triu_t, in_=triu_h[:])
            ident_t = setup.tile([128, 128], F32)
            nc.sync.dma_start(out=ident_t, in_=ident_h[:])

            # small hot constants via the Act queue; the big masks/triu are
            # dehoisted below the qk section so their ~330 KB of small
            # descriptors don't round-robin against the xT chunks at t=0
            ones_col = setup.tile([128, 1], F32)
            nc.scalar.dma_start(out=ones_col, in_=ones_col_h[:])
            ones_row = setup.tile([1, 128], F32)
            nc.scalar.dma_start(out=ones_row, in_=ones_row_h[:])
            ones_cb = setup.tile([64, 1], BF16)
            nc.scalar.dma_start(out=ones_cb, in_=ones_cb_h[:])
            nones_cb = setup.tile([64, 1], BF16)
            nc.scalar.dma_start(out=nones_cb, in_=nones_cb_h[:])

            # preload the Sigmoid activation table during the idle head
            eps_t = setup.tile([128, 1], F32)
            c0_t = setup.tile([128, 1], F32)
            with tc.high_priority():
                nc.vector.memset(eps_t, 1e-9)
                nc.vector.memset(c0_t, C0)
            warm_t = setup.tile([1, 1], F32)
            with tc.high_priority():
                nc.scalar.activation(warm_t, eps_t[0:1, 0:1], AF.Sigmoid)

            # bf16 copies of x/W for the qk matmuls (DVE is idle this early)
            xb_t = setup.tile([128, 4, S], BF16)
            wb_t = setup.tile([128, 4, P2], BF16)
            with tc.high_priority():
                for c in range(4):
                    nc.vector.tensor_copy(xb_t[:, c, :], xT_t[:, c, :])
                nc.vector.tensor_copy(wb_t, wT_t)

            # ---------------- qT/kT = (x @ W.T).T halves  [64, S] ----------
            # band products are staged into [64, 1026] rows with col c
            # holding index i = c-1 (cols 0, 1024, 1025 are don't-care pads
            # feeding only masked/unused lanes).
            qT_t = setup.tile([64, S], F32)
            kT_t = setup.tile([64, S], F32)
            # bf16 product rows: the 48 banded-diff matmuls use 128-col
            # slices of these as lhsT - bf16 gets FWL (4x faster LDWEIGHTS)
            tu_t = setup.tile([64, 1026], BF16)
            tl_t = setup.tile([64, 1026], BF16)
            ps_diff = psd.tile([128, 3, NB], F32)
            with tc.high_priority():
                # pads: col c holds score index c-1.  s_u[1023] and s_l[0]
                # must be -inf-ish so the edge rows' one-neighbor softmax
                # saturates (a_u[0]=1, a_l[1023]=1); the MM sums 64 copies,
                # still hugely negative.  Cols 0/1025 feed only unused lanes.
                NEG = -1e30
                nc.vector.memset(tu_t[:, 0:1], 0.0)
                nc.vector.memset(tu_t[:, 1024:1025], NEG)
                nc.vector.memset(tu_t[:, 1025:1026], 0.0)
                nc.vector.memset(tl_t[:, 0:1], 0.0)
                nc.vector.memset(tl_t[:, 1:2], NEG)
                nc.vector.memset(tl_t[:, 1025:1026], 0.0)
                for j in range(2):
                    for half, dest_t in enumerate((qT_t, kT_t)):
                        ps = mm.tile([64, 512], F32, tag="mmbig")
                        for c in range(4):
                            nc.tensor.matmul(
                                ps[:],
                                lhsT=wb_t[:, c, half * 64:(half + 1) * 64],
                                rhs=xb_t[:, c, j * 512:(j + 1) * 512],
                                start=(c == 0),
                                stop=(c == 3),
                            )
                        nc.vector.tensor_scalar_add(
                            dest_t[:, j * 512:(j + 1) * 512], ps,
                            bias_t[half * 64:(half + 1) * 64, 0:1])
                    # band products, staged so col c holds score index c-1:
                    #   tu[1+i] = s_u[i] = q_i.k_{i+1}     (i in [0,1023))
                    #   tl[2+i] = s_l[i+1] = q_{i+1}.k_i   -> tl[c] = s_l[c-1]
                    lo, hi = (0, 511) if j == 0 else (511, 1023)
                    nc.vector.tensor_mul(tu_t[:, 1 + lo:1 + hi],
                                         qT_t[:, lo:hi],
                                         kT_t[:, lo + 1:hi + 1])
                    nc.vector.tensor_mul(tl_t[:, 2 + lo:2 + hi],
                                         qT_t[:, lo + 1:hi + 1],
                                         kT_t[:, lo:hi])

                # banded score diffs straight into [128, 3, NB] tile layout:
                # ps_diff[p, g, r] = s_u[i+g-1] - s_l[i+g-1],  i = 128r + p,
                # via paired matmuls: (tu2 slice).T @ ones + (tl2 slice).T @
                # -ones accumulated into one PSUM column.
                for g in range(3):
                    for r in range(NB):
                        c0 = 128 * r + g
                        nc.tensor.matmul(ps_diff[:, g, r:r + 1],
                                         lhsT=tu_t[:, c0:c0 + 128],
                                         rhs=ones_cb,
                                         start=True, stop=False)
                        nc.tensor.matmul(ps_diff[:, g, r:r + 1],
                                         lhsT=tl_t[:, c0:c0 + 128],
                                         rhs=nones_cb,
                                         start=False, stop=True)

                # 2-element softmax via sigmoid on all 3 offset groups at once
                a_u = setup.tile([128, 3, NB], F32)
                nc.scalar.activation(a_u, ps_diff, AF.Sigmoid, scale=1.0 / E)
                a_l = setup.tile([128, 3, NB], F32)
                nc.scalar.activation(a_l, ps_diff, AF.Sigmoid, scale=-1.0 / E)

                # g_l[i] = g_u[i-1] = sqrt(a_u[i-1]*a_l[i] + eps)  (cols 0:8)
                # g_u[i]            = sqrt(a_u[i]*a_l[i+1] + eps)  (cols 8:16)
                gq_t = setup.tile([128, 2, NB], F32)
                nc.vector.tensor_mul(gq_t, _ap(a_u[:], 0, [[24, 128], [8, 2], [1, NB]]),
                                     _ap(a_l[:], 8, [[24, 128], [8, 2], [1, NB]]))
                g_t = setup.tile([128, 2, NB], F32)
                nc.scalar.activation(g_t, gq_t, AF.Sqrt, bias=eps_t[:, 0:1])

                # na band values as per-row vectors:
                #   na_b2[:,0,:] = na[i,i-1] = g_l + pr_l*(1-g_l)
                #   na_b2[:,1,:] = na[i,i+1] = g_u + pr_u*(1-g_u)
                omg2 = setup.tile([128, 2, NB], F32)
                nc.vector.tensor_scalar(omg2, g_t, -1.0, 1.0, op0=ALU.mult,
                                        op1=ALU.add)
                prm = setup.tile([128, 2, NB], F32)
                nc.vector.tensor_mul(prm, pr_lu, omg2)
                # lnin: col 1 = na[i,i+1], col 0 = nd = na[i,i], so ONE Ln
                # produces both u and ln(nd) (avoids an extra ACT table swap)
                lnin = setup.tile([128, 2, NB], F32)
                nc.vector.tensor_add(lnin[:, 1, :], prm[:, 1, :], g_t[:, 1, :])
                nc.vector.tensor_scalar(lnin[:, 0, :], pr_d, 1.0 - C0, C0,
                                        op0=ALU.mult, op1=ALU.add)
                nd_t = lnin[:, 0, :]
                u2 = setup.tile([128, 2, NB], F32)
                nc.scalar.activation(u2, lnin, AF.Ln, bias=eps_t[:, 0:1])
                u_t = u2[:, 1, :]
                lnnd = u2[:, 0, :]
                # preload the Exp table; gated on u2 so it cannot be hoisted
                # between the Sigmoid/Sqrt/Ln uses and thrash the table RAM
                nc.scalar.activation(warm_t, u2[0:1, 0, 0:1], AF.Exp,
                                     scale=-1.0)

                # band-correction scalars vs the bulk na formula:
                #   na_band - na_bulk = (g - C0) * (1 - prior)  at [i, i-/+1]
                pu1_2 = setup.tile([128, 2, NB], F32)
                nc.vector.tensor_scalar(pu1_2, pr_lu, -1.0, 1.0, op0=ALU.mult,
                                        op1=ALU.add)
                gc_2 = setup.tile([128, 2, NB], F32)
                nc.vector.tensor_scalar(gc_2, g_t, C0, None, op0=ALU.subtract)
                cu_2 = setup.tile([128, 2, NB], F32)
                nc.vector.tensor_mul(cu_2, gc_2, pu1_2)

                # ---- U = exclusive prefix sum of u ----
                inc_ps = ps_small.tile([128, NB], F32, tag="tiny")
                nc.tensor.matmul(inc_ps, lhsT=triu_t, rhs=u_t, start=True,
                                 stop=True)
                exc_t = setup.tile([128, NB], F32)
                nc.vector.tensor_sub(exc_t, inc_ps, u_t)

                cs_ps = ps_small.tile([1, NB], F32, tag="tiny")  # block sums
                nc.tensor.matmul(cs_ps, lhsT=ones_col, rhs=u_t, start=True,
                                 stop=True)
                bp_t = setup.tile([1, NB], F32)
                nc.vector.memset(bp_t[:, 0:1], 0.0)
                nc.vector.tensor_copy(bp_t[:, 1:8], cs_ps[0:1, 0:7])
                zer_t = setup.tile([1, NB], F32)
                nc.vector.memset(zer_t, 0.0)
                bpx_t = setup.tile([1, NB], F32)      # exclusive block prefix
                nc.vector.tensor_tensor_scan(bpx_t, bp_t, zer_t, 0.0,
                                             op0=ALU.add, op1=ALU.add)
                bpr_ps = ps_small.tile([128, NB], F32, tag="tiny")
                nc.tensor.matmul(bpr_ps, lhsT=ones_row, rhs=bpx_t, start=True,
                                 stop=True)
                U_t = setup.tile([128, NB], F32)
                nc.vector.tensor_add(U_t, exc_t, bpr_ps)
                negU_t = setup.tile([128, NB], F32)
                nc.vector.tensor_scalar(negU_t, U_t, -1.0, None, op0=ALU.mult)

                # U -> row layout via identity-matmul transpose (no DMA):
                # out[0, n] = sum_k U[k, r] * I[k, n] = U[n, r]
                ur_ps = psu.tile([1, S], F32, tag="urow")
                for r in range(NB):
                    nc.tensor.matmul(ur_ps[0:1, r * 128:(r + 1) * 128],
                                     lhsT=U_t[:, r:r + 1], rhs=ident_t,
                                     start=True, stop=True)
                U_lin = setup.tile([1, S], F32)
                nc.scalar.activation(U_lin, ur_ps, AF.Identity)
                Ur_ps = psrep.tile([128, S], F32, tag="urep")
                for lo in (0, 512):
                    nc.tensor.matmul(Ur_ps[:, lo:lo + 512], lhsT=ones_row,
                                     rhs=U_lin[0:1, lo:lo + 512], start=True,
                                     stop=True)

            # ---------------- pass 1: na full rows ----------
            for r in range(NB):
                na_t = napool.tile([128, S], F32, tag="na")
                nc.vector.tensor_scalar(na_t, pr_ts[r], 1.0 - C0, C0,
                                        op0=ALU.mult, op1=ALU.add)
                nc.sync.dma_start(out=na_out[r * 128:(r + 1) * 128, :],
                                  in_=na_t)

            # ---------------- pass 2: c_attn rows ----------------
            # cd = |U[j] - U[i]|.  U is non-increasing (u < 0), so left of
            # the diagonal window d >= 0 and right of it d <= 0: one
            # tensor_scalar per region gives |d| directly; a true abs (max
            # of +/-d) is only needed in the 130-wide diagonal window.
            # The diag is pre-patched so exp(-cd) lands na[i,i] there.
            for r in range(NB):
                w0 = r * 128 - 1
                wlo = max(w0, 0)
                whi = min(w0 + 130, S)
                wd = whi - wlo
                mo = wlo - w0
                Ui = U_t[:, r:r + 1]

                # exp(-|d|) fused into ONE ACT pass per region: the affine
                # lives in the activation's scale/bias, so no cd
                # intermediate for left/right.  DVE only builds the 130-wide
                # ambiguous mid window (true abs + diag patch).
                c2_t = c2pool.tile([128, S], F32, tag="c2")
                if wlo > 0:
                    # j < i: |d| = Ur - U_i -> exp(U_i - Ur)
                    nc.scalar.activation(c2_t[:, 0:wlo], Ur_ps[:, 0:wlo],
                                         AF.Exp, scale=-1.0, bias=Ui)
                if whi < S:
                    # j > i: |d| = U_i - Ur -> exp(Ur - U_i)
                    nc.scalar.activation(c2_t[:, whi:S], Ur_ps[:, whi:S],
                                         AF.Exp, scale=1.0,
                                         bias=negU_t[:, r:r + 1])
                ta_t = cdpool.tile([128, 130], F32, tag="ta")
                nc.vector.tensor_scalar(ta_t[:, :wd], Ur_ps[:, wlo:whi],
                                        Ui, None, op0=ALU.subtract)
                tb_t = cdpool.tile([128, 130], F32, tag="tb")
                nc.vector.tensor_scalar(tb_t[:, :wd], Ur_ps[:, wlo:whi],
                                        Ui, -1.0, op0=ALU.subtract,
                                        op1=ALU.mult)
                t5w = cdpool.tile([128, 130], F32, tag="t5w")
                nc.vector.tensor_scalar(t5w[:, :wd], md_t[:, mo:mo + wd],
                                        lnnd[:, r:r + 1], None, op0=ALU.mult)
                cdm_t = cdpool.tile([128, 130], F32, tag="cdm")
                nc.vector.tensor_max(cdm_t[:, :wd], ta_t[:, :wd],
                                     tb_t[:, :wd])
                nc.vector.tensor_sub(cdm_t[:, :wd], cdm_t[:, :wd],
                                     t5w[:, :wd])
                nc.scalar.activation(c2_t[:, wlo:whi], cdm_t[:, :wd],
                                     AF.Exp, scale=-1.0)
                nc.scalar.dma_start(out=c_out[r * 128:(r + 1) * 128, :],
                                    in_=c2_t)

            # ---------------- band overwrite (tail; tiny stores) ----------
            # rebuild the [128,130] window from pr_t with the off-diagonal
            # corrections added, stored as 130-wide rows (520 B/descriptor).
            # Same SP ring as the bulk na stores -> FIFO gives WAW order.
            for r in range(NB):
                w0 = r * 128 - 1
                wlo = max(w0, 0)
                whi = min(w0 + 130, S)
                wd = whi - wlo
                mo = wlo - w0
                bw_t = napool.tile([128, 130], F32, tag="bw")
                nc.vector.tensor_scalar(bw_t[:, :wd], pr_ts[r][:, wlo:whi],
                                        1.0 - C0, C0, op0=ALU.mult,
                                        op1=ALU.add)
                t1w = napool.tile([128, 130], F32, tag="t1w")
                nc.vector.tensor_scalar(t1w[:, :wd], mu_t[:, mo:mo + wd],
                                        cu_2[:, 1, r:r + 1], None,
                                        op0=ALU.mult)
                nc.vector.tensor_add(bw_t[:, :wd], bw_t[:, :wd], t1w[:, :wd])
                t2w = napool.tile([128, 130], F32, tag="t2w")
                nc.vector.tensor_scalar(t2w[:, :wd], ml_t[:, mo:mo + wd],
                                        cu_2[:, 0, r:r + 1], None,
                                        op0=ALU.mult)
                nc.vector.tensor_add(bw_t[:, :wd], bw_t[:, :wd], t2w[:, :wd])
                nc.sync.dma_start(
                    out=_ap(na_out, r * 128 * S + wlo, [[S, 128], [1, wd]]),
                    in_=bw_t[:, :wd])

    _split_multi_waits(nc)
    return nc


def _get_nc():
    if "nc" not in _CACHE:
        _CACHE["nc"] = build_nc()
    return _CACHE["nc"]


def run(inputs, trace=False, tmpdir=None):
    nc = _get_nc()
    context = np.asarray(inputs["context"], np.float32)
    prior = np.asarray(inputs["prior"], np.float32)
    w = np.asarray(inputs["proj_weight"], np.float32)
    bias = np.asarray(inputs["proj_bias"], np.float32)

    wT = np.ascontiguousarray(w.T)                     # [E, 2P]
    bcol = np.ascontiguousarray(bias.reshape(P2, 1))
    in_maps = []
    for b in range(B):
        in_maps.append({
            "xT": np.ascontiguousarray(context[:, b, :].T),   # [E, S]
            "wT": wT,
            "bvec": bcol,
            "prior": np.ascontiguousarray(prior[b]),
        })
    try:
        res = run_bass_kernel_spmd(nc, in_maps, list(range(B)), trace=trace,
                                   tmpdir=tmpdir)
    except ModuleNotFoundError:
        res = run_bass_kernel_spmd(nc, in_maps, list(range(B)), trace=False)
    c = np.stack([res.results[i]["c_out"] for i in range(B)])
    na = np.stack([res.results[i]["na_out"] for i in range(B)])
    return (c, na), res


def kernel(**inputs):
    (c, na), _ = run(inputs)
    return (c, na)
